# revision 23
# baseline (speedup 1.0000x reference)
"""AttnBlock2D (GroupNorm + QKV 1x1 + full self-attention over N=4096 + proj +
residual) on 8 Trainium2 NeuronCores.

Sharding: data-parallel over the 4 (b*t) frames x 2-way query split within each
frame (core i -> frame i//2, query half i%2).  Each core receives its frame with
tokens rotated so its own query half is tokens [0:2048] (softmax/PV are invariant
to key permutation), so a single uniform SPMD program runs on all 8 cores.

GroupNorm is folded into the QKV weights: hn[c,n] = a_c*x[c,n] + b_c, with the
per-channel affine (a, b) computed from global group stats obtained via a tiny
(32,2) AllReduce of per-core partial sums.  The attention scale C**-0.5 is folded
into wq.  All heavy matmuls run in bf16 with fp32 PSUM accumulation; the residual
add is done in fp32, so bf16 rounding only touches the small attention branch.
"""

import numpy as np
import ml_dtypes

import concourse.bass as bass
import concourse.bacc as bacc
import concourse.mybir as mybir
import concourse.tile as tile
from concourse.bass_utils import run_bass_kernel_spmd

F32 = mybir.dt.float32
BF16 = mybir.dt.bfloat16
FP8 = mybir.dt.float8e4
AF = mybir.ActivationFunctionType
ALU = mybir.AluOpType

# Problem shape (hardcoded per contract)
B, C, T, H, W = 1, 512, 4, 64, 64
N = H * W                # 4096 tokens per frame
GROUPS = 32
EPS = 1e-6
NC = 8                   # cores
NQ = N // 2              # queries per core (2048)
CB = C // 128            # channel blocks (4)
GN_COUNT = (C // GROUPS) * T * N   # elements per group = 16*4*4096

# fp8 weight rescale: folded q/k/v weights (~2e-3) sit below the fp8e4m3
# normal range, so scale them x32 and divide out RS^2=1024 inside the exp
# (S) and RS inside the PV normalization -- exact powers of two.
RS = 32.0

_CACHED = {}


def _t(pool, shape, dtype, nm, bufs=None):
    """pool.tile with name==tag (each call site gets its own persistent slot)."""
    return pool.tile(shape, dtype, name=nm, tag=nm, bufs=bufs)



def _build(debug=False, ablate=()):
    nc = bacc.Bacc(num_devices=NC, name="attnblock2d")
    dbg = {}
    def dbg_out(name, ap):
        if not debug:
            return
        t = nc.dram_tensor(f"dbg_{name}", tuple(ap.shape), ap.dtype,
                           kind="ExternalOutput")
        nc.sync.dma_start(out=t[tuple(slice(0, s) for s in ap.shape)], in_=ap)

    xb_d = nc.dram_tensor("xb", (C, N), FP8, kind="ExternalInput")
    xh_d = nc.dram_tensor("xh", (C, NQ), F32, kind="ExternalInput")
    w_d = {
        "q": nc.dram_tensor("wq", (C, C), BF16, kind="ExternalInput"),
        "k": nc.dram_tensor("wk", (C, C), BF16, kind="ExternalInput"),
        "v": nc.dram_tensor("wv", (C, C), BF16, kind="ExternalInput"),
        "p": nc.dram_tensor("wp", (C, C), BF16, kind="ExternalInput"),
    }
    vec_d = {
        name: nc.dram_tensor(name, (C,), F32, kind="ExternalInput")
        for name in ("gamma", "beta", "bq", "bk", "bv", "bp")
    }
    gmap_d = nc.dram_tensor("gmap", (C, GROUPS), F32, kind="ExternalInput")
    gscat_d = nc.dram_tensor("gscat", (GROUPS, C), F32, kind="ExternalInput")
    identb_d = nc.dram_tensor("identb", (128, 128), BF16, kind="ExternalInput")
    yf = nc.dram_tensor("yf", (C, NQ), F32, kind="ExternalOutput")

    scale = float(C) ** -0.5

    with tile.TileContext(nc) as tc:
        with (
            tc.tile_pool(name="singles", bufs=1) as singles,
            tc.tile_pool(name="xown", bufs=1) as xown_p,
            tc.tile_pool(name="kp", bufs=1) as k_p,
            tc.tile_pool(name="vp", bufs=1) as v_p,
            tc.tile_pool(name="qp", bufs=1) as q_p,
            tc.tile_pool(name="wfold", bufs=1) as wfold_p,
            tc.tile_pool(name="psmm", bufs=2, space="PSUM") as ps_mm,
            tc.tile_pool(name="pstr", bufs=1, space="PSUM") as ps_tr,
            tc.tile_pool(name="pssm", bufs=1, space="PSUM") as ps_sm,
            tc.tile_pool(name="dram", bufs=1, space="DRAM") as dram_p,
        ):
            # ---------------- phase 0: input DMAs (critical-path order) -----
            # xown feeds stats -> AllReduce (the longest dependency chain);
            # identb + weights feed the PE transposes that fill the wait.
            xown = [_t(xown_p, [128, NQ], F32, f'xown_{b}') for b in range(CB)]
            for b in range(CB):
                for sg in range(4):
                    nc.sync.dma_start(
                        out=xown[b][:, 512 * sg:512 * (sg + 1)],
                        in_=xh_d[128 * b:128 * (b + 1), 512 * sg:512 * (sg + 1)])

            identb = _t(singles, [128, 128], BF16, 'identb')
            nc.scalar.dma_start(out=identb, in_=identb_d[:, :])

            gmap = _t(singles, [128, CB, GROUPS], F32, 'gmap')
            nc.scalar.dma_start(
                out=gmap, in_=gmap_d[:, :].rearrange("(b p) g -> p b g", p=128))
            gscat = _t(singles, [GROUPS, CB, 128], F32, 'gscat')
            nc.scalar.dma_start(
                out=gscat, in_=gscat_d[:, :].rearrange("g (b c) -> g b c", c=128))

            vecs = {}
            for name, ten in vec_d.items():
                t = _t(singles, [128, CB], F32, f'vec_{name}')
                nc.scalar.dma_start(out=t, in_=ten[:].rearrange("(b p) -> p b", p=128))
                vecs[name] = t


            # folded (transposed, bf16) weights live for the whole kernel
            wTp = {
                name: [_t(wfold_p, [128, C], BF16, f'wTp_{name}{b}')
                       for b in range(CB)]
                for name in ("q", "k", "v", "p")
            }

            with (
                tc.tile_pool(name="xb16p", bufs=1) as xb16_p,
                tc.tile_pool(name="setup", bufs=1) as setup,
            ):
                # full frame cast to bf16 (gpsimd casting DMA)
                x8 = [_t(xb16_p, [128, 2, N], FP8, f'x8_{ch}')
                      for ch in range(2)]
                for ch in range(2):
                    nc.sync.dma_start(
                        out=x8[ch],
                        in_=xb_d[256 * ch:256 * (ch + 1), :].rearrange(
                            "(h p) n -> p h n", p=128))

                # weights (bf16, o rows on partitions), transposed early so
                # the PE does this during the DMA/stats/collective window.
                # NOTE: the rhs of a transpose-mode matmul must be a true
                # identity matrix (its nonzero structure routes the data).
                wTu = {"p": wTp["p"]}
                for name in ("p", "q", "k", "v"):
                    ten = w_d[name]
                    wbig = setup.tile([128, CB, C], BF16, tag="wnat", bufs=2)
                    nc.scalar.dma_start(
                        out=wbig,
                        in_=ten[:, :].rearrange("(b p) c -> p b c", p=128))
                    if name != "p":
                        wTu[name] = [_t(setup, [128, C], BF16, f'wTu_{name}{b}')
                                     for b in range(CB)]
                    for cb in range(CB):
                        pw = ps_tr.tile([128, CB, 128], BF16, tag="tr")
                        for ob in range(CB):
                            nc.tensor.matmul(
                                pw[:, ob, :],
                                wbig[:, ob, 128 * cb:128 * (cb + 1)],
                                identb[:, :], is_transpose=True)
                        nc.scalar.copy(out=wTu[name][cb],
                                       in_=pw.rearrange("p a b -> p (a b)"))

                # ---------------- phase 1: groupnorm partial stats ----------
                partials = []
                for b in range(CB):
                    st6 = _t(setup, [128, 4, 6], F32, f'st6_{b}')
                    xv = xown[b].rearrange("p (a f) -> p a f", f=512)
                    for sg in range(4):
                        nc.vector.bn_stats(out=st6[:, sg, :], in_=xv[:, sg, :])
                    mv = _t(setup, [128, 2], F32, f'mv_{b}')
                    nc.vector.bn_aggr(out=mv, in_=st6)
                    # partial = [sum, sumsq] = [mean*nq, (var+mean^2)*nq]
                    part = _t(setup, [128, 2], F32, f'part_{b}')
                    sq = _t(setup, [128, 1], F32, f'sq_{b}')
                    nc.scalar.activation(out=sq, in_=mv[:, 0:1], func=AF.Square)
                    nc.vector.tensor_tensor(out=sq, in0=sq, in1=mv[:, 1:2],
                                            op=ALU.add)
                    nc.scalar.mul(out=part[:, 0:1], in_=mv[:, 0:1], mul=float(NQ))
                    nc.scalar.mul(out=part[:, 1:2], in_=sq, mul=float(NQ))
                    partials.append(part)

                psg = ps_sm.tile([GROUPS, 2], F32, tag="sm")
                for b in range(CB):
                    nc.tensor.matmul(psg[:, :], gmap[:, b, :], partials[b][:, :],
                                     start=(b == 0), stop=(b == CB - 1))
                part_g = _t(setup, [GROUPS, 2], F32, 'part_g')
                nc.vector.tensor_copy(out=part_g, in_=psg)
                dbg_out('part_g', part_g)

                # ---------------- phase 2: AllReduce ------------------------
                cin = _t(dram_p, [GROUPS, 2], F32, 'cin')
                cout = _t(dram_p, [GROUPS, 2], F32, 'cout')
                gl = _t(setup, [GROUPS, 2], F32, 'gl')
                if "nocoll" in ablate:
                    nc.scalar.mul(out=gl, in_=part_g, mul=float(NC))
                else:
                    nc.gpsimd.dma_start(out=cin[:], in_=part_g)
                    nc.gpsimd.collective_compute(
                        "AllReduce", ALU.add,
                        replica_groups=[list(range(NC))],
                        ins=[cin.opt()], outs=[cout.opt()])
                    nc.gpsimd.dma_start(out=gl, in_=cout[:])
                dbg_out('gl', gl)

                # ---------------- phase 3: stats -> per-channel affine ------
                musd = _t(setup, [GROUPS, 2], F32, 'musd')  # [mu, rstd] per group
                inv_n = 1.0 / float(GN_COUNT)
                nc.scalar.mul(out=musd[:, 0:1], in_=gl[:, 0:1], mul=inv_n)
                m2 = _t(setup, [GROUPS, 1], F32, 'm2')
                nc.scalar.mul(out=m2, in_=gl[:, 1:2], mul=inv_n)
                musq = _t(setup, [GROUPS, 1], F32, 'musq')
                nc.scalar.activation(out=musq, in_=musd[:, 0:1], func=AF.Square)
                nc.vector.tensor_tensor(out=m2, in0=m2, in1=musq, op=ALU.subtract)
                epst = _t(setup, [GROUPS, 1], F32, 'epst')
                nc.vector.memset(epst, EPS)
                nc.scalar.activation(out=m2, in_=m2, func=AF.Sqrt, bias=epst)
                nc.vector.reciprocal(out=musd[:, 1:2], in_=m2)
                dbg_out('musd', musd)

                # scatter group stats to channels; per-channel affine a, b
                a_by_w = {"q": [], "k": [], "v": []}
                bvec16 = []
                for b in range(CB):
                    pssc = ps_sm.tile([128, 2], F32, tag="sm")
                    nc.tensor.matmul(pssc[:, :], gscat[:, b, :], musd[:, :],
                                     start=True, stop=True)
                    mc = _t(setup, [128, 2], F32, f'mc_{b}')
                    nc.vector.tensor_copy(out=mc, in_=pssc)
                    a = _t(setup, [128, 1], F32, f'a_{b}')
                    nc.vector.tensor_tensor(out=a, in0=mc[:, 1:2],
                                            in1=vecs["gamma"][:, b:b + 1],
                                            op=ALU.mult)
                    bb = _t(setup, [128, 1], F32, f'bb_{b}')
                    nc.vector.tensor_tensor(out=bb, in0=mc[:, 0:1], in1=a,
                                            op=ALU.mult)
                    nc.vector.tensor_tensor(out=bb, in0=vecs["beta"][:, b:b + 1],
                                            in1=bb, op=ALU.subtract)
                    bv16 = _t(setup, [128, 1], BF16, f'bv16_{b}')
                    nc.vector.tensor_copy(out=bv16, in_=bb)
                    bvec16.append(bv16)
                    asq = _t(setup, [128, 1], F32, f'asq_{b}')
                    nc.scalar.mul(out=asq, in_=a, mul=scale * RS)
                    ar = _t(setup, [128, 1], F32, f'ar_{b}')
                    nc.scalar.mul(out=ar, in_=a, mul=RS)
                    a_by_w["q"].append(asq)
                    a_by_w["k"].append(ar)
                    a_by_w["v"].append(ar)

                # fold q/k/v weights to fp8 DoubleRow layout: RS * a * wT
                wTp8 = {name: [_t(wfold_p, [128, 2, C], FP8, f'wTp8_{name}{ch}')
                               for ch in range(2)]
                        for name in ("q", "k", "v")}
                for name in ("q", "k", "v"):
                    for b in range(CB):
                        nc.vector.tensor_scalar_mul(
                            wTp8[name][b // 2][:, b % 2, :], wTu[name][b],
                            a_by_w[name][b])

                # folded biases biasF_w[o] = s*RS*((w @ b)[o] + bias_w[o]) from
                # the unfolded bf16 weights (a cancels against b = beta - mu*a)
                biasF = {}
                for name, bvec, s in (("q", "bq", scale * RS),
                                      ("k", "bk", RS), ("v", "bv", 1.0)):
                    bf_t = _t(singles, [128, CB], F32, f'biasF_{name}')
                    for ob in range(CB):
                        psb = ps_sm.tile([128, 1], F32, tag="sm")
                        for b in range(CB):
                            nc.tensor.matmul(
                                psb[:, :],
                                wTu[name][b][:, 128 * ob:128 * (ob + 1)],
                                bvec16[b][:, :],
                                start=(b == 0), stop=(b == CB - 1))
                        nc.vector.tensor_scalar(
                            out=bf_t[:, ob:ob + 1], in0=psb,
                            scalar1=vecs[bvec][:, ob:ob + 1], scalar2=s,
                            op0=ALU.add, op1=ALU.mult)
                    biasF[name] = bf_t

                # v bias folds into the projection bias: since sum_j p_j/d = 1,
                # out = wp@(ov + bias_v) + bp = proj(ov) + (wp@bias_v + bp)
                bvF16 = []
                for b in range(CB):
                    t16 = _t(setup, [128, 1], BF16, f'bvF16_{b}')
                    nc.vector.tensor_copy(out=t16, in_=biasF["v"][:, b:b + 1])
                    bvF16.append(t16)
                biasFP = _t(singles, [128, CB], F32, 'biasFP')
                for ob in range(CB):
                    psb = ps_sm.tile([128, 1], F32, tag="sm")
                    for b in range(CB):
                        nc.tensor.matmul(
                            psb[:, :],
                            wTp["p"][b][:, 128 * ob:128 * (ob + 1)],
                            bvF16[b][:, :],
                            start=(b == 0), stop=(b == CB - 1))
                    nc.vector.tensor_tensor(
                        out=biasFP[:, ob:ob + 1], in0=psb,
                        in1=vecs["bp"][:, ob:ob + 1], op=ALU.add)

                # ---------------- phase 4: K, V^T, Q ------------------------
                K_sb = [_t(k_p, [128, 2, N], FP8, f'K_{oh}')
                        for oh in range(2)]
                for ob in range(CB):
                    for jc in range(N // 512):
                        pk = ps_mm.tile([128, 512], F32, tag="mm")
                        for ch in range(2):
                            nc.tensor.matmul(
                                pk[:, :],
                                wTp8["k"][ch][:, :, 128 * ob:128 * (ob + 1)],
                                x8[ch][:, :, 512 * jc:512 * (jc + 1)],
                                perf_mode=mybir.MatmulPerfMode.DoubleRow,
                                start=(ch == 0), stop=(ch == 1))
                        nc.vector.tensor_scalar_add(
                            K_sb[ob // 2][:, ob % 2, 512 * jc:512 * (jc + 1)],
                            pk, biasF["k"][:, ob:ob + 1])

                Q_sb = [_t(q_p, [128, 2, NQ], FP8, f'Q_{oh}')
                        for oh in range(2)]
                for ob in range(CB):
                    for ic in range(NQ // 512):
                        pq = ps_mm.tile([128, 512], F32, tag="mm")
                        for ch in range(2):
                            nc.tensor.matmul(
                                pq[:, :],
                                wTp8["q"][ch][:, :, 128 * ob:128 * (ob + 1)],
                                x8[ch][:, :, 512 * ic:512 * (ic + 1)],
                                perf_mode=mybir.MatmulPerfMode.DoubleRow,
                                start=(ch == 0), stop=(ch == 1))
                        nc.vector.tensor_scalar_add(
                            Q_sb[ob // 2][:, ob % 2, 512 * ic:512 * (ic + 1)],
                            pq, biasF["q"][:, ob:ob + 1])

                V_sb = [_t(v_p, [128, 2, C], FP8, f'V_{j2}')
                        for j2 in range(N // 256)]
                for jb in range(N // 128):
                    pv = ps_mm.tile([128, 512], F32, tag="mm")
                    for ch in range(2):
                        nc.tensor.matmul(
                            pv[:, :], x8[ch][:, :, 128 * jb:128 * (jb + 1)],
                            wTp8["v"][ch][:, :, :],
                            perf_mode=mybir.MatmulPerfMode.DoubleRow,
                            start=(ch == 0), stop=(ch == 1))
                    nc.vector.tensor_copy(out=V_sb[jb // 2][:, jb % 2, :], in_=pv)


            if "noattn" in ablate:
                for ob in range(CB):
                    nc.sync.dma_start(out=yf[128 * ob:128 * (ob + 1), :],
                                      in_=xown[ob])
                nc.compile_marker = True
            # ---------------- phase 5: attention ----------------------------
            skip_attn = "noattn" in ablate
            with (
                tc.tile_pool(name="attn", bufs=1) as attn_p,
                tc.tile_pool(name="pbuf", bufs=2) as p_pool,
                tc.tile_pool(name="ptbuf", bufs=2) as pt_pool,
                tc.tile_pool(name="obuf", bufs=3) as o_pool,
            ):
                AO = _t(attn_p, [128, CB, NQ], BF16, 'AO')   # attn out (c, i) blocks
                NIB = 0 if skip_attn else NQ // 128      # 16 query blocks
                reps = 4 if "rep4" in ablate else 1
                for rep, ib in __import__("itertools").product(range(reps), range(NIB)):
                    P_sb = p_pool.tile([128, N], BF16, tag="P")
                    dparts = o_pool.tile([128, N // 1024], F32, tag="dp")
                    for jc4 in range(N // 1024):
                        pss = ps_mm.tile([128, 2, 512], F32, tag="s2", bufs=2)
                        for half in range(2):
                            jc = 2 * jc4 + half
                            for oh in range(2):
                                nc.tensor.matmul(
                                    pss[:, half, :],
                                    Q_sb[oh][:, :, 128 * ib:128 * (ib + 1)],
                                    K_sb[oh][:, :, 512 * jc:512 * (jc + 1)],
                                    perf_mode=mybir.MatmulPerfMode.DoubleRow,
                                    start=(oh == 0), stop=(oh == 1))
                        nc.scalar.activation(
                            out=P_sb[:, 1024 * jc4:1024 * (jc4 + 1)],
                            in_=pss.rearrange("p a b -> p (a b)"),
                            func=AF.Exp, scale=1.0 / (RS * RS),
                            accum_out=dparts[:, jc4:jc4 + 1])
                    dsum = o_pool.tile([128, 1], F32, tag="ds")
                    nc.vector.reduce_sum(out=dsum, in_=dparts,
                                         axis=mybir.AxisListType.X)
                    nc.scalar.mul(out=dsum, in_=dsum, mul=RS)
                    rinv = o_pool.tile([128, 1], F32, tag="ri")
                    nc.vector.reciprocal(out=rinv, in_=dsum)

                    # transpose P in 128x128 blocks on the (otherwise idle)
                    # DMA engines, straight into PT
                    PT = pt_pool.tile([128, N // 128, 128], BF16, tag="PT")
                    for jb in range(N // 128):
                        nc.sync.dma_start(out=PT[:, jb, :],
                                          in_=P_sb[:, 128 * jb:128 * (jb + 1)],
                                          transpose=True)

                    # cast PT to fp8 on the (idle) SWDGE path, in 4 chunks
                    # so PV can start before the last transposes land
                    PT8 = pt_pool.tile([128, N // 128, 128], FP8, tag="PT8", bufs=3)
                    for qt in range(4):
                        nc.gpsimd.dma_start(out=PT8[:, 8 * qt:8 * (qt + 1), :],
                                            in_=PT[:, 8 * qt:8 * (qt + 1), :])

                    # PV: out^T (i, c) accumulated over j; then scale by 1/d
                    pso = ps_mm.tile([128, 512], F32, tag="mm")
                    NJ2 = N // 256
                    for j2 in range(NJ2):
                        nc.tensor.matmul(pso[:, :],
                                         PT8[:, 2 * j2:2 * j2 + 2, :],
                                         V_sb[j2][:, :, :],
                                         perf_mode=mybir.MatmulPerfMode.DoubleRow,
                                         start=(j2 == 0), stop=(j2 == NJ2 - 1))
                    OT = o_pool.tile([128, C], BF16, tag="OT")
                    nc.vector.tensor_scalar_mul(OT, pso, rinv)

                    # transpose out^T back to (c, i) into AO via DMA
                    for cb in range(CB):
                        nc.sync.dma_start(out=AO[:, cb, 128 * ib:128 * (ib + 1)],
                                          in_=OT[:, 128 * cb:128 * (cb + 1)],
                                          transpose=True)

                # ------------- phase 6: proj + residual + store -------------
                for rep, ob in __import__("itertools").product(
                        range(1 if skip_attn else (4 if "rep4" in ablate else 1)),
                        () if skip_attn else range(CB)):
                    for ic in range(NQ // 512):
                        psp = ps_mm.tile([128, 512], F32, tag="mm")
                        for b in range(CB):
                            nc.tensor.matmul(
                                psp[:, :],
                                wTp["p"][b][:, 128 * ob:128 * (ob + 1)],
                                AO[:, b, 512 * ic:512 * (ic + 1)],
                                start=(b == 0), stop=(b == CB - 1))
                        ot = o_pool.tile([128, 512], F32, tag="out")
                        nc.scalar.activation(out=ot, in_=psp, func=AF.Identity,
                                             bias=biasFP[:, ob:ob + 1])
                        nc.vector.tensor_tensor(
                            out=ot, in0=ot,
                            in1=xown[ob][:, 512 * ic:512 * (ic + 1)], op=ALU.add)
                        nc.sync.dma_start(
                            out=yf[128 * ob:128 * (ob + 1),
                                   512 * ic:512 * (ic + 1)],
                            in_=ot)

    nc.compile()
    return nc


def _get_nc(debug=False, ablate=()):
    key = f"nc{int(debug)}{sorted(ablate)}"
    if key not in _CACHED:
        _CACHED[key] = _build(debug, ablate)
    return _CACHED[key]


def _host_inputs(x, gamma, beta, wq, bq, wk, bk, wv, bv, wp, bp):
    gmap = np.zeros((C, GROUPS), dtype=np.float32)
    gmap[np.arange(C), np.arange(C) // (C // GROUPS)] = 1.0
    gscat = np.ascontiguousarray(gmap.T)
    identb = np.eye(128, dtype=ml_dtypes.bfloat16)

    shared = {
        "wq": np.ascontiguousarray(np.asarray(wq, np.float32).astype(ml_dtypes.bfloat16)),
        "wk": np.ascontiguousarray(np.asarray(wk, np.float32).astype(ml_dtypes.bfloat16)),
        "wv": np.ascontiguousarray(np.asarray(wv, np.float32).astype(ml_dtypes.bfloat16)),
        "wp": np.ascontiguousarray(np.asarray(wp, np.float32).astype(ml_dtypes.bfloat16)),
        "gamma": np.ascontiguousarray(gamma, np.float32),
        "beta": np.ascontiguousarray(beta, np.float32),
        "bq": np.ascontiguousarray(bq, np.float32),
        "bk": np.ascontiguousarray(bk, np.float32),
        "bv": np.ascontiguousarray(bv, np.float32),
        "bp": np.ascontiguousarray(bp, np.float32),
        "gmap": gmap, "gscat": gscat, "identb": identb,
    }
    in_maps = []
    for core in range(NC):
        f, h = core // 2, core % 2
        frame = np.asarray(x[0, :, f], dtype=np.float32).reshape(C, N)
        if h == 1:
            frame = np.concatenate([frame[:, NQ:], frame[:, :NQ]], axis=1)
        m = dict(shared)
        m["xb"] = np.ascontiguousarray(frame.astype(ml_dtypes.float8_e4m3))
        m["xh"] = np.ascontiguousarray(frame[:, :NQ])
        in_maps.append(m)
    return in_maps


def _assemble(results):
    y = np.empty((B, C, T, H, W), dtype=np.float32)
    for core in range(NC):
        f, h = core // 2, core % 2
        part = results[core]["yf"].reshape(C, NQ // W, W)
        rows = slice(0, H // 2) if h == 0 else slice(H // 2, H)
        y[0, :, f, rows, :] = part
    return y


def kernel(x, gamma, beta, wq, bq, wk, bk, wv, bv, wp, bp):
    nc = _get_nc()
    in_maps = _host_inputs(x, gamma, beta, wq, bq, wk, bk, wv, bv, wp, bp)
    res = run_bass_kernel_spmd(nc, in_maps, core_ids=list(range(NC)))
    return _assemble(res.results)


# revision 26
# speedup vs baseline: 4.7708x; 4.7708x over previous
"""AttnBlock2D (GroupNorm + QKV 1x1 + full self-attention over N=4096 + proj +
residual) on 8 Trainium2 NeuronCores.

Sharding: data-parallel over the 4 (b*t) frames x 2-way query split within each
frame (core i -> frame i//2, query half i%2).  Each core receives its frame with
tokens rotated so its own query half is tokens [0:2048] (softmax/PV are invariant
to key permutation), so a single uniform SPMD program runs on all 8 cores.

GroupNorm is folded into the QKV weights: hn[c,n] = a_c*x[c,n] + b_c, with the
per-channel affine (a, b) computed from global group stats obtained via a tiny
(32,2) AllReduce of per-core partial sums.  The attention scale C**-0.5 is folded
into wq.  All heavy matmuls run in bf16 with fp32 PSUM accumulation; the residual
add is done in fp32, so bf16 rounding only touches the small attention branch.
"""

import numpy as np
import ml_dtypes

import concourse.bass as bass
import concourse.bacc as bacc
import concourse.mybir as mybir
import concourse.tile as tile
from concourse.bass_utils import run_bass_kernel_spmd

F32 = mybir.dt.float32
BF16 = mybir.dt.bfloat16
FP8 = mybir.dt.float8e4
AF = mybir.ActivationFunctionType
ALU = mybir.AluOpType

# Problem shape (hardcoded per contract)
B, C, T, H, W = 1, 512, 4, 64, 64
N = H * W                # 4096 tokens per frame
GROUPS = 32
EPS = 1e-6
NC = 8                   # cores
NQ = N // 2              # queries per core (2048)
CB = C // 128            # channel blocks (4)
GN_COUNT = (C // GROUPS) * T * N   # elements per group = 16*4*4096

# fp8 weight rescale: folded q/k/v weights (~2e-3) sit below the fp8e4m3
# normal range, so scale them x32 and divide out RS^2=1024 inside the exp
# (S) and RS inside the PV normalization -- exact powers of two.
RS = 32.0

_CACHED = {}


def _t(pool, shape, dtype, nm, bufs=None):
    """pool.tile with name==tag (each call site gets its own persistent slot)."""
    return pool.tile(shape, dtype, name=nm, tag=nm, bufs=bufs)



def _build(debug=False, ablate=()):
    nc = bacc.Bacc(num_devices=NC, name="attnblock2d")
    dbg = {}
    def dbg_out(name, ap):
        if not debug:
            return
        t = nc.dram_tensor(f"dbg_{name}", tuple(ap.shape), ap.dtype,
                           kind="ExternalOutput")
        nc.sync.dma_start(out=t[tuple(slice(0, s) for s in ap.shape)], in_=ap)

    xb_d = nc.dram_tensor("xb", (C, N), FP8, kind="ExternalInput")
    xh_d = nc.dram_tensor("xh", (C, NQ), F32, kind="ExternalInput")
    w_d = {
        "q": nc.dram_tensor("wq", (C, C), BF16, kind="ExternalInput"),
        "k": nc.dram_tensor("wk", (C, C), BF16, kind="ExternalInput"),
        "v": nc.dram_tensor("wv", (C, C), BF16, kind="ExternalInput"),
        "p": nc.dram_tensor("wp", (C, C), BF16, kind="ExternalInput"),
    }
    vec_d = {
        name: nc.dram_tensor(name, (C,), F32, kind="ExternalInput")
        for name in ("gamma", "beta", "bq", "bk", "bv", "bp")
    }
    gmap_d = nc.dram_tensor("gmap", (C, GROUPS), F32, kind="ExternalInput")
    gscat_d = nc.dram_tensor("gscat", (GROUPS, C), F32, kind="ExternalInput")
    identb_d = nc.dram_tensor("identb", (128, 128), BF16, kind="ExternalInput")
    yf = nc.dram_tensor("yf", (C, NQ), F32, kind="ExternalOutput")

    scale = float(C) ** -0.5

    with tile.TileContext(nc) as tc:
        with (
            tc.tile_pool(name="singles", bufs=1) as singles,
            tc.tile_pool(name="xown", bufs=1) as xown_p,
            tc.tile_pool(name="kp", bufs=1) as k_p,
            tc.tile_pool(name="vp", bufs=1) as v_p,
            tc.tile_pool(name="qp", bufs=1) as q_p,
            tc.tile_pool(name="wfold", bufs=1) as wfold_p,
            tc.tile_pool(name="psmm", bufs=2, space="PSUM") as ps_mm,
            tc.tile_pool(name="pstr", bufs=1, space="PSUM") as ps_tr,
            tc.tile_pool(name="pssm", bufs=1, space="PSUM") as ps_sm,
            tc.tile_pool(name="dram", bufs=1, space="DRAM") as dram_p,
        ):
            # ---------------- phase 0: input DMAs (critical-path order) -----
            # xown feeds stats -> AllReduce (the longest dependency chain);
            # identb + weights feed the PE transposes that fill the wait.
            xown = [_t(xown_p, [128, NQ], F32, f'xown_{b}') for b in range(CB)]
            for b in range(CB):
                for sg in range(4):
                    nc.sync.dma_start(
                        out=xown[b][:, 512 * sg:512 * (sg + 1)],
                        in_=xh_d[128 * b:128 * (b + 1), 512 * sg:512 * (sg + 1)])

            identb = _t(singles, [128, 128], BF16, 'identb')
            nc.scalar.dma_start(out=identb, in_=identb_d[:, :])
            ident8 = _t(singles, [128, 128], FP8, 'ident8')
            nc.vector.tensor_copy(out=ident8, in_=identb)

            gmap = _t(singles, [128, CB, GROUPS], F32, 'gmap')
            nc.scalar.dma_start(
                out=gmap, in_=gmap_d[:, :].rearrange("(b p) g -> p b g", p=128))
            gscat = _t(singles, [GROUPS, CB, 128], F32, 'gscat')
            nc.scalar.dma_start(
                out=gscat, in_=gscat_d[:, :].rearrange("g (b c) -> g b c", c=128))

            vecs = {}
            for name, ten in vec_d.items():
                t = _t(singles, [128, CB], F32, f'vec_{name}')
                nc.scalar.dma_start(out=t, in_=ten[:].rearrange("(b p) -> p b", p=128))
                vecs[name] = t


            # folded (transposed, bf16) weights live for the whole kernel
            wTp = {
                name: [_t(wfold_p, [128, C], BF16, f'wTp_{name}{b}')
                       for b in range(CB)]
                for name in ("q", "k", "v", "p")
            }

            with (
                tc.tile_pool(name="xb16p", bufs=1) as xb16_p,
                tc.tile_pool(name="setup", bufs=1) as setup,
            ):
                # full frame cast to bf16 (gpsimd casting DMA)
                x8 = [_t(xb16_p, [128, 2, N], FP8, f'x8_{ch}')
                      for ch in range(2)]
                for ch in range(2):
                    nc.sync.dma_start(
                        out=x8[ch],
                        in_=xb_d[256 * ch:256 * (ch + 1), :].rearrange(
                            "(h p) n -> p h n", p=128))

                # weights (bf16, o rows on partitions), transposed early so
                # the PE does this during the DMA/stats/collective window.
                # NOTE: the rhs of a transpose-mode matmul must be a true
                # identity matrix (its nonzero structure routes the data).
                wTu = {"p": wTp["p"]}
                for name in ("p", "q", "k", "v"):
                    ten = w_d[name]
                    wbig = setup.tile([128, CB, C], BF16, tag="wnat", bufs=2)
                    nc.scalar.dma_start(
                        out=wbig,
                        in_=ten[:, :].rearrange("(b p) c -> p b c", p=128))
                    if name != "p":
                        wTu[name] = [_t(setup, [128, C], BF16, f'wTu_{name}{b}')
                                     for b in range(CB)]
                    for cb in range(CB):
                        pw = ps_tr.tile([128, CB, 128], BF16, tag="tr")
                        for ob in range(CB):
                            nc.tensor.matmul(
                                pw[:, ob, :],
                                wbig[:, ob, 128 * cb:128 * (cb + 1)],
                                identb[:, :], is_transpose=True)
                        nc.scalar.copy(out=wTu[name][cb],
                                       in_=pw.rearrange("p a b -> p (a b)"))

                # ---------------- phase 1: groupnorm partial stats ----------
                partials = []
                for b in range(CB):
                    st6 = _t(setup, [128, 4, 6], F32, f'st6_{b}')
                    xv = xown[b].rearrange("p (a f) -> p a f", f=512)
                    for sg in range(4):
                        nc.vector.bn_stats(out=st6[:, sg, :], in_=xv[:, sg, :])
                    mv = _t(setup, [128, 2], F32, f'mv_{b}')
                    nc.vector.bn_aggr(out=mv, in_=st6)
                    # partial = [sum, sumsq] = [mean*nq, (var+mean^2)*nq]
                    part = _t(setup, [128, 2], F32, f'part_{b}')
                    sq = _t(setup, [128, 1], F32, f'sq_{b}')
                    nc.scalar.activation(out=sq, in_=mv[:, 0:1], func=AF.Square)
                    nc.vector.tensor_tensor(out=sq, in0=sq, in1=mv[:, 1:2],
                                            op=ALU.add)
                    nc.scalar.mul(out=part[:, 0:1], in_=mv[:, 0:1], mul=float(NQ))
                    nc.scalar.mul(out=part[:, 1:2], in_=sq, mul=float(NQ))
                    partials.append(part)

                psg = ps_sm.tile([GROUPS, 2], F32, tag="sm")
                for b in range(CB):
                    nc.tensor.matmul(psg[:, :], gmap[:, b, :], partials[b][:, :],
                                     start=(b == 0), stop=(b == CB - 1))
                part_g = _t(setup, [GROUPS, 2], F32, 'part_g')
                nc.vector.tensor_copy(out=part_g, in_=psg)
                dbg_out('part_g', part_g)

                # ---------------- phase 2: AllReduce ------------------------
                cin = _t(dram_p, [GROUPS, 2], F32, 'cin')
                cout = _t(dram_p, [GROUPS, 2], F32, 'cout')
                gl = _t(setup, [GROUPS, 2], F32, 'gl')
                if "nocoll" in ablate:
                    nc.scalar.mul(out=gl, in_=part_g, mul=float(NC))
                else:
                    nc.gpsimd.dma_start(out=cin[:], in_=part_g)
                    nc.gpsimd.collective_compute(
                        "AllReduce", ALU.add,
                        replica_groups=[list(range(NC))],
                        ins=[cin.opt()], outs=[cout.opt()])
                    nc.gpsimd.dma_start(out=gl, in_=cout[:])
                dbg_out('gl', gl)

                # ---------------- phase 3: stats -> per-channel affine ------
                musd = _t(setup, [GROUPS, 2], F32, 'musd')  # [mu, rstd] per group
                inv_n = 1.0 / float(GN_COUNT)
                nc.scalar.mul(out=musd[:, 0:1], in_=gl[:, 0:1], mul=inv_n)
                m2 = _t(setup, [GROUPS, 1], F32, 'm2')
                nc.scalar.mul(out=m2, in_=gl[:, 1:2], mul=inv_n)
                musq = _t(setup, [GROUPS, 1], F32, 'musq')
                nc.scalar.activation(out=musq, in_=musd[:, 0:1], func=AF.Square)
                nc.vector.tensor_tensor(out=m2, in0=m2, in1=musq, op=ALU.subtract)
                epst = _t(setup, [GROUPS, 1], F32, 'epst')
                nc.vector.memset(epst, EPS)
                nc.scalar.activation(out=m2, in_=m2, func=AF.Sqrt, bias=epst)
                nc.vector.reciprocal(out=musd[:, 1:2], in_=m2)
                dbg_out('musd', musd)

                # scatter group stats to channels; per-channel affine a, b
                a_by_w = {"q": [], "k": [], "v": []}
                bvec16 = []
                for b in range(CB):
                    pssc = ps_sm.tile([128, 2], F32, tag="sm")
                    nc.tensor.matmul(pssc[:, :], gscat[:, b, :], musd[:, :],
                                     start=True, stop=True)
                    mc = _t(setup, [128, 2], F32, f'mc_{b}')
                    nc.vector.tensor_copy(out=mc, in_=pssc)
                    a = _t(setup, [128, 1], F32, f'a_{b}')
                    nc.vector.tensor_tensor(out=a, in0=mc[:, 1:2],
                                            in1=vecs["gamma"][:, b:b + 1],
                                            op=ALU.mult)
                    bb = _t(setup, [128, 1], F32, f'bb_{b}')
                    nc.vector.tensor_tensor(out=bb, in0=mc[:, 0:1], in1=a,
                                            op=ALU.mult)
                    nc.vector.tensor_tensor(out=bb, in0=vecs["beta"][:, b:b + 1],
                                            in1=bb, op=ALU.subtract)
                    bv16 = _t(setup, [128, 1], BF16, f'bv16_{b}')
                    nc.vector.tensor_copy(out=bv16, in_=bb)
                    bvec16.append(bv16)
                    asq = _t(setup, [128, 1], F32, f'asq_{b}')
                    nc.scalar.mul(out=asq, in_=a, mul=scale * RS)
                    ar = _t(setup, [128, 1], F32, f'ar_{b}')
                    nc.scalar.mul(out=ar, in_=a, mul=RS)
                    a_by_w["q"].append(asq)
                    a_by_w["k"].append(ar)
                    a_by_w["v"].append(ar)

                # fold q/k/v weights to fp8 DoubleRow layout: RS * a * wT
                wTp8 = {name: [_t(wfold_p, [128, 2, C], FP8, f'wTp8_{name}{ch}')
                               for ch in range(2)]
                        for name in ("q", "k", "v")}
                for name in ("q", "k", "v"):
                    for b in range(CB):
                        nc.vector.tensor_scalar_mul(
                            wTp8[name][b // 2][:, b % 2, :], wTu[name][b],
                            a_by_w[name][b])

                # folded biases biasF_w[o] = s*RS*((w @ b)[o] + bias_w[o]) from
                # the unfolded bf16 weights (a cancels against b = beta - mu*a)
                biasF = {}
                for name, bvec, s in (("q", "bq", scale * RS),
                                      ("k", "bk", RS), ("v", "bv", 1.0)):
                    bf_t = _t(singles, [128, CB], F32, f'biasF_{name}')
                    for ob in range(CB):
                        psb = ps_sm.tile([128, 1], F32, tag="sm")
                        for b in range(CB):
                            nc.tensor.matmul(
                                psb[:, :],
                                wTu[name][b][:, 128 * ob:128 * (ob + 1)],
                                bvec16[b][:, :],
                                start=(b == 0), stop=(b == CB - 1))
                        nc.vector.tensor_scalar(
                            out=bf_t[:, ob:ob + 1], in0=psb,
                            scalar1=vecs[bvec][:, ob:ob + 1], scalar2=s,
                            op0=ALU.add, op1=ALU.mult)
                    biasF[name] = bf_t

                # v bias folds into the projection bias: since sum_j p_j/d = 1,
                # out = wp@(ov + bias_v) + bp = proj(ov) + (wp@bias_v + bp)
                bvF16 = []
                for b in range(CB):
                    t16 = _t(setup, [128, 1], BF16, f'bvF16_{b}')
                    nc.vector.tensor_copy(out=t16, in_=biasF["v"][:, b:b + 1])
                    bvF16.append(t16)
                biasFP = _t(singles, [128, CB], F32, 'biasFP')
                for ob in range(CB):
                    psb = ps_sm.tile([128, 1], F32, tag="sm")
                    for b in range(CB):
                        nc.tensor.matmul(
                            psb[:, :],
                            wTp["p"][b][:, 128 * ob:128 * (ob + 1)],
                            bvF16[b][:, :],
                            start=(b == 0), stop=(b == CB - 1))
                    nc.vector.tensor_tensor(
                        out=biasFP[:, ob:ob + 1], in0=psb,
                        in1=vecs["bp"][:, ob:ob + 1], op=ALU.add)

                # ---------------- phase 4: K, V^T, Q ------------------------
                K_sb = [_t(k_p, [128, 2, N], FP8, f'K_{oh}')
                        for oh in range(2)]
                for ob in range(CB):
                    for jc in range(N // 512):
                        pk = ps_mm.tile([128, 512], F32, tag="mm")
                        for ch in range(2):
                            nc.tensor.matmul(
                                pk[:, :],
                                wTp8["k"][ch][:, :, 128 * ob:128 * (ob + 1)],
                                x8[ch][:, :, 512 * jc:512 * (jc + 1)],
                                perf_mode=mybir.MatmulPerfMode.DoubleRow,
                                start=(ch == 0), stop=(ch == 1))
                        nc.vector.tensor_scalar_add(
                            K_sb[ob // 2][:, ob % 2, 512 * jc:512 * (jc + 1)],
                            pk, biasF["k"][:, ob:ob + 1])

                Q_sb = [_t(q_p, [128, 2, NQ], FP8, f'Q_{oh}')
                        for oh in range(2)]
                for ob in range(CB):
                    for ic in range(NQ // 512):
                        pq = ps_mm.tile([128, 512], F32, tag="mm")
                        for ch in range(2):
                            nc.tensor.matmul(
                                pq[:, :],
                                wTp8["q"][ch][:, :, 128 * ob:128 * (ob + 1)],
                                x8[ch][:, :, 512 * ic:512 * (ic + 1)],
                                perf_mode=mybir.MatmulPerfMode.DoubleRow,
                                start=(ch == 0), stop=(ch == 1))
                        nc.vector.tensor_scalar_add(
                            Q_sb[ob // 2][:, ob % 2, 512 * ic:512 * (ic + 1)],
                            pq, biasF["q"][:, ob:ob + 1])

                V_sb = [_t(v_p, [128, 2, C], FP8, f'V_{j2}')
                        for j2 in range(N // 256)]
                for jb in range(N // 128):
                    pv = ps_mm.tile([128, 512], F32, tag="mm")
                    for ch in range(2):
                        nc.tensor.matmul(
                            pv[:, :], x8[ch][:, :, 128 * jb:128 * (jb + 1)],
                            wTp8["v"][ch][:, :, :],
                            perf_mode=mybir.MatmulPerfMode.DoubleRow,
                            start=(ch == 0), stop=(ch == 1))
                    nc.vector.tensor_copy(out=V_sb[jb // 2][:, jb % 2, :], in_=pv)


            if "noattn" in ablate:
                for ob in range(CB):
                    nc.sync.dma_start(out=yf[128 * ob:128 * (ob + 1), :],
                                      in_=xown[ob])
                nc.compile_marker = True
            # ---------------- phase 5: attention ----------------------------
            skip_attn = "noattn" in ablate
            with (
                tc.tile_pool(name="attn", bufs=1) as attn_p,
                tc.tile_pool(name="pbuf", bufs=2) as p_pool,
                tc.tile_pool(name="ptbuf", bufs=2) as pt_pool,
                tc.tile_pool(name="obuf", bufs=3) as o_pool,
            ):
                AO = _t(attn_p, [128, CB, NQ], BF16, 'AO')   # attn out (c, i) blocks
                NIB = 0 if skip_attn else NQ // 128      # 16 query blocks
                reps = 4 if "rep4" in ablate else 1
                petr = "dmatr" not in ablate
                for rep, ib in __import__("itertools").product(range(reps), range(NIB)):
                    P_sb = p_pool.tile([128, N], BF16, tag="P")
                    dparts = o_pool.tile([128, N // 1024], F32, tag="dp")
                    for jc4 in range(N // 1024):
                        pss = ps_mm.tile([128, 2, 512], F32, tag="s2", bufs=2)
                        for half in range(2):
                            jc = 2 * jc4 + half
                            for oh in range(2):
                                nc.tensor.matmul(
                                    pss[:, half, :],
                                    Q_sb[oh][:, :, 128 * ib:128 * (ib + 1)],
                                    K_sb[oh][:, :, 512 * jc:512 * (jc + 1)],
                                    perf_mode=mybir.MatmulPerfMode.DoubleRow,
                                    start=(oh == 0), stop=(oh == 1))
                        nc.scalar.activation(
                            out=P_sb[:, 1024 * jc4:1024 * (jc4 + 1)],
                            in_=pss.rearrange("p a b -> p (a b)"),
                            func=AF.Exp, scale=1.0 / (RS * RS),
                            accum_out=dparts[:, jc4:jc4 + 1])
                    dsum = o_pool.tile([128, 1], F32, tag="ds")
                    nc.vector.reduce_sum(out=dsum, in_=dparts,
                                         axis=mybir.AxisListType.X)
                    nc.scalar.mul(out=dsum, in_=dsum, mul=RS)
                    rinv = o_pool.tile([128, 1], F32, tag="ri")
                    nc.vector.reciprocal(out=rinv, in_=dsum)

                    PT8 = pt_pool.tile([128, N // 128, 128], FP8, tag="PT8", bufs=3)
                    if petr:
                        # PE transposes of bf16 P, 8 packed per PSUM bank; the
                        # fp8 cast rides along on the PSUM->SBUF copy
                        for rnd in range(4):
                            ptp = ps_tr.tile([128, 8, 128], BF16, tag="tr")
                            for t8 in range(8):
                                jb = 8 * rnd + t8
                                nc.tensor.matmul(
                                    ptp[:, t8, :],
                                    P_sb[:, 128 * jb:128 * (jb + 1)],
                                    identb[:, :], is_transpose=True)
                            if rnd % 2 == 0:
                                nc.vector.tensor_copy(
                                    out=PT8[:, 8 * rnd:8 * rnd + 8, :], in_=ptp)
                            else:
                                nc.scalar.copy(
                                    out=PT8[:, 8 * rnd:8 * rnd + 8, :], in_=ptp)
                    else:
                        # transpose P in 128x128 blocks on the DMA engines
                        PT = pt_pool.tile([128, N // 128, 128], BF16, tag="PT")
                        for jb in range(N // 128):
                            nc.sync.dma_start(out=PT[:, jb, :],
                                              in_=P_sb[:, 128 * jb:128 * (jb + 1)],
                                              transpose=True)
                        if "dvecast" in ablate:
                            for qt in range(4):
                                nc.vector.tensor_copy(
                                    out=PT8[:, 8 * qt:8 * (qt + 1), :],
                                    in_=PT[:, 8 * qt:8 * (qt + 1), :])
                        else:
                            # cast PT to fp8 on the SWDGE path, in 4 chunks
                            for qt in range(4):
                                nc.gpsimd.dma_start(
                                    out=PT8[:, 8 * qt:8 * (qt + 1), :],
                                    in_=PT[:, 8 * qt:8 * (qt + 1), :])

                    # PV: out^T (i, c) accumulated over j; then scale by 1/d
                    pso = ps_mm.tile([128, 512], F32, tag="mm")
                    NJ2 = N // 256
                    for j2 in range(NJ2):
                        nc.tensor.matmul(pso[:, :],
                                         PT8[:, 2 * j2:2 * j2 + 2, :],
                                         V_sb[j2][:, :, :],
                                         perf_mode=mybir.MatmulPerfMode.DoubleRow,
                                         start=(j2 == 0), stop=(j2 == NJ2 - 1))
                    OT = o_pool.tile([128, C], BF16, tag="OT")
                    nc.vector.tensor_scalar_mul(OT, pso, rinv)

                    if petr:
                        pt2 = ps_tr.tile([128, CB, 128], BF16, tag="tr")
                        for cb in range(CB):
                            nc.tensor.matmul(pt2[:, cb, :],
                                             OT[:, 128 * cb:128 * (cb + 1)],
                                             identb[:, :], is_transpose=True)
                        nc.scalar.copy(out=AO[:, :, 128 * ib:128 * (ib + 1)],
                                       in_=pt2)
                    else:
                        # transpose out^T back to (c, i) into AO via DMA
                        for cb in range(CB):
                            nc.sync.dma_start(
                                out=AO[:, cb, 128 * ib:128 * (ib + 1)],
                                in_=OT[:, 128 * cb:128 * (cb + 1)],
                                transpose=True)

                # ------------- phase 6: proj + residual + store -------------
                for rep, ob in __import__("itertools").product(
                        range(1 if skip_attn else (4 if "rep4" in ablate else 1)),
                        () if skip_attn else range(CB)):
                    for ic in range(NQ // 512):
                        psp = ps_mm.tile([128, 512], F32, tag="mm")
                        for b in range(CB):
                            nc.tensor.matmul(
                                psp[:, :],
                                wTp["p"][b][:, 128 * ob:128 * (ob + 1)],
                                AO[:, b, 512 * ic:512 * (ic + 1)],
                                start=(b == 0), stop=(b == CB - 1))
                        ot = o_pool.tile([128, 512], F32, tag="out")
                        nc.scalar.activation(out=ot, in_=psp, func=AF.Identity,
                                             bias=biasFP[:, ob:ob + 1])
                        nc.vector.tensor_tensor(
                            out=ot, in0=ot,
                            in1=xown[ob][:, 512 * ic:512 * (ic + 1)], op=ALU.add)
                        nc.sync.dma_start(
                            out=yf[128 * ob:128 * (ob + 1),
                                   512 * ic:512 * (ic + 1)],
                            in_=ot)

    nc.compile()
    return nc


def _get_nc(debug=False, ablate=()):
    key = f"nc{int(debug)}{sorted(ablate)}"
    if key not in _CACHED:
        _CACHED[key] = _build(debug, ablate)
    return _CACHED[key]


def _host_inputs(x, gamma, beta, wq, bq, wk, bk, wv, bv, wp, bp):
    gmap = np.zeros((C, GROUPS), dtype=np.float32)
    gmap[np.arange(C), np.arange(C) // (C // GROUPS)] = 1.0
    gscat = np.ascontiguousarray(gmap.T)
    identb = np.eye(128, dtype=ml_dtypes.bfloat16)

    shared = {
        "wq": np.ascontiguousarray(np.asarray(wq, np.float32).astype(ml_dtypes.bfloat16)),
        "wk": np.ascontiguousarray(np.asarray(wk, np.float32).astype(ml_dtypes.bfloat16)),
        "wv": np.ascontiguousarray(np.asarray(wv, np.float32).astype(ml_dtypes.bfloat16)),
        "wp": np.ascontiguousarray(np.asarray(wp, np.float32).astype(ml_dtypes.bfloat16)),
        "gamma": np.ascontiguousarray(gamma, np.float32),
        "beta": np.ascontiguousarray(beta, np.float32),
        "bq": np.ascontiguousarray(bq, np.float32),
        "bk": np.ascontiguousarray(bk, np.float32),
        "bv": np.ascontiguousarray(bv, np.float32),
        "bp": np.ascontiguousarray(bp, np.float32),
        "gmap": gmap, "gscat": gscat, "identb": identb,
    }
    in_maps = []
    for core in range(NC):
        f, h = core // 2, core % 2
        frame = np.asarray(x[0, :, f], dtype=np.float32).reshape(C, N)
        if h == 1:
            frame = np.concatenate([frame[:, NQ:], frame[:, :NQ]], axis=1)
        m = dict(shared)
        m["xb"] = np.ascontiguousarray(frame.astype(ml_dtypes.float8_e4m3))
        m["xh"] = np.ascontiguousarray(frame[:, :NQ])
        in_maps.append(m)
    return in_maps


def _assemble(results):
    y = np.empty((B, C, T, H, W), dtype=np.float32)
    for core in range(NC):
        f, h = core // 2, core % 2
        part = results[core]["yf"].reshape(C, NQ // W, W)
        rows = slice(0, H // 2) if h == 0 else slice(H // 2, H)
        y[0, :, f, rows, :] = part
    return y


def kernel(x, gamma, beta, wq, bq, wk, bk, wv, bv, wp, bp):
    nc = _get_nc()
    in_maps = _host_inputs(x, gamma, beta, wq, bq, wk, bk, wv, bv, wp, bp)
    res = run_bass_kernel_spmd(nc, in_maps, core_ids=list(range(NC)))
    return _assemble(res.results)


# revision 27
# speedup vs baseline: 7.2035x; 1.5099x over previous
"""AttnBlock2D (GroupNorm + QKV 1x1 + full self-attention over N=4096 + proj +
residual) on 8 Trainium2 NeuronCores.

Sharding: data-parallel over the 4 (b*t) frames x 2-way query split within each
frame (core i -> frame i//2, query half i%2).  Each core receives its frame with
tokens rotated so its own query half is tokens [0:2048] (softmax/PV are invariant
to key permutation), so a single uniform SPMD program runs on all 8 cores.

GroupNorm is folded into the QKV weights: hn[c,n] = a_c*x[c,n] + b_c, with the
per-channel affine (a, b) computed from global group stats obtained via a tiny
(32,2) AllReduce of per-core partial sums.  The attention scale C**-0.5 is folded
into wq.  All heavy matmuls run in bf16 with fp32 PSUM accumulation; the residual
add is done in fp32, so bf16 rounding only touches the small attention branch.
"""

import numpy as np
import ml_dtypes

import concourse.bass as bass
import concourse.bacc as bacc
import concourse.mybir as mybir
import concourse.tile as tile
from concourse.bass_utils import run_bass_kernel_spmd

F32 = mybir.dt.float32
BF16 = mybir.dt.bfloat16
FP8 = mybir.dt.float8e4
AF = mybir.ActivationFunctionType
ALU = mybir.AluOpType

# Problem shape (hardcoded per contract)
B, C, T, H, W = 1, 512, 4, 64, 64
N = H * W                # 4096 tokens per frame
GROUPS = 32
EPS = 1e-6
NC = 8                   # cores
NQ = N // 2              # queries per core (2048)
CB = C // 128            # channel blocks (4)
GN_COUNT = (C // GROUPS) * T * N   # elements per group = 16*4*4096

# fp8 weight rescale: folded q/k/v weights (~2e-3) sit below the fp8e4m3
# normal range, so scale them x32 and divide out RS^2=1024 inside the exp
# (S) and RS inside the PV normalization -- exact powers of two.
RS = 32.0

_CACHED = {}


def _t(pool, shape, dtype, nm, bufs=None):
    """pool.tile with name==tag (each call site gets its own persistent slot)."""
    return pool.tile(shape, dtype, name=nm, tag=nm, bufs=bufs)



def _build(debug=False, ablate=()):
    nc = bacc.Bacc(num_devices=NC, name="attnblock2d")
    dbg = {}
    def dbg_out(name, ap):
        if not debug:
            return
        t = nc.dram_tensor(f"dbg_{name}", tuple(ap.shape), ap.dtype,
                           kind="ExternalOutput")
        nc.sync.dma_start(out=t[tuple(slice(0, s) for s in ap.shape)], in_=ap)

    xb_d = nc.dram_tensor("xb", (C, N), FP8, kind="ExternalInput")
    xh_d = nc.dram_tensor("xh", (C, NQ), F32, kind="ExternalInput")
    w_d = {
        "q": nc.dram_tensor("wq", (C, C), BF16, kind="ExternalInput"),
        "k": nc.dram_tensor("wk", (C, C), BF16, kind="ExternalInput"),
        "v": nc.dram_tensor("wv", (C, C), BF16, kind="ExternalInput"),
        "p": nc.dram_tensor("wp", (C, C), BF16, kind="ExternalInput"),
    }
    vec_d = {
        name: nc.dram_tensor(name, (C,), F32, kind="ExternalInput")
        for name in ("gamma", "beta", "bq", "bk", "bv", "bp")
    }
    gmap_d = nc.dram_tensor("gmap", (C, GROUPS), F32, kind="ExternalInput")
    gscat_d = nc.dram_tensor("gscat", (GROUPS, C), F32, kind="ExternalInput")
    identb_d = nc.dram_tensor("identb", (128, 128), BF16, kind="ExternalInput")
    yf = nc.dram_tensor("yf", (C, NQ), F32, kind="ExternalOutput")

    scale = float(C) ** -0.5

    with tile.TileContext(nc) as tc:
        with (
            tc.tile_pool(name="singles", bufs=1) as singles,
            tc.tile_pool(name="xown", bufs=1) as xown_p,
            tc.tile_pool(name="kp", bufs=1) as k_p,
            tc.tile_pool(name="vp", bufs=1) as v_p,
            tc.tile_pool(name="qp", bufs=1) as q_p,
            tc.tile_pool(name="wfold", bufs=1) as wfold_p,
            tc.tile_pool(name="psmm", bufs=2, space="PSUM") as ps_mm,
            tc.tile_pool(name="pstr", bufs=2, space="PSUM") as ps_tr,
            tc.tile_pool(name="dram", bufs=1, space="DRAM") as dram_p,
        ):
            # ---------------- phase 0: input DMAs (critical-path order) -----
            # xown feeds stats -> AllReduce (the longest dependency chain);
            # identb + weights feed the PE transposes that fill the wait.
            xown = [_t(xown_p, [128, NQ], F32, f'xown_{b}') for b in range(CB)]
            for b in range(CB):
                for sg in range(4):
                    nc.sync.dma_start(
                        out=xown[b][:, 512 * sg:512 * (sg + 1)],
                        in_=xh_d[128 * b:128 * (b + 1), 512 * sg:512 * (sg + 1)])

            identb = _t(singles, [128, 128], BF16, 'identb')
            nc.scalar.dma_start(out=identb, in_=identb_d[:, :])
            ident8 = _t(singles, [128, 128], FP8, 'ident8')
            nc.vector.tensor_copy(out=ident8, in_=identb)

            gmap = _t(singles, [128, CB, GROUPS], F32, 'gmap')
            nc.scalar.dma_start(
                out=gmap, in_=gmap_d[:, :].rearrange("(b p) g -> p b g", p=128))
            gscat = _t(singles, [GROUPS, CB, 128], F32, 'gscat')
            nc.scalar.dma_start(
                out=gscat, in_=gscat_d[:, :].rearrange("g (b c) -> g b c", c=128))

            vecs = {}
            for name, ten in vec_d.items():
                t = _t(singles, [128, CB], F32, f'vec_{name}')
                nc.scalar.dma_start(out=t, in_=ten[:].rearrange("(b p) -> p b", p=128))
                vecs[name] = t


            # folded (transposed, bf16) weights live for the whole kernel
            wTp = {
                name: [_t(wfold_p, [128, C], BF16, f'wTp_{name}{b}')
                       for b in range(CB)]
                for name in ("q", "k", "v", "p")
            }

            with (
                tc.tile_pool(name="xb16p", bufs=1) as xb16_p,
                tc.tile_pool(name="setup", bufs=1) as setup,
            ):
                # full frame cast to bf16 (gpsimd casting DMA)
                x8 = [_t(xb16_p, [128, 2, N], FP8, f'x8_{ch}')
                      for ch in range(2)]
                for ch in range(2):
                    nc.sync.dma_start(
                        out=x8[ch],
                        in_=xb_d[256 * ch:256 * (ch + 1), :].rearrange(
                            "(h p) n -> p h n", p=128))

                # weights (bf16, o rows on partitions), transposed early so
                # the PE does this during the DMA/stats/collective window.
                # NOTE: the rhs of a transpose-mode matmul must be a true
                # identity matrix (its nonzero structure routes the data).
                wTu = {"p": wTp["p"]}
                for name in ("p", "q", "k", "v"):
                    ten = w_d[name]
                    wbig = setup.tile([128, CB, C], BF16, tag="wnat", bufs=2)
                    nc.scalar.dma_start(
                        out=wbig,
                        in_=ten[:, :].rearrange("(b p) c -> p b c", p=128))
                    if name != "p":
                        wTu[name] = [_t(setup, [128, C], BF16, f'wTu_{name}{b}')
                                     for b in range(CB)]
                    for cb in range(CB):
                        pw = ps_tr.tile([128, CB, 128], BF16, tag="tr")
                        for ob in range(CB):
                            nc.tensor.matmul(
                                pw[:, ob, :],
                                wbig[:, ob, 128 * cb:128 * (cb + 1)],
                                identb[:, :], is_transpose=True)
                        nc.scalar.copy(out=wTu[name][cb],
                                       in_=pw.rearrange("p a b -> p (a b)"))

                # ---------------- phase 1: groupnorm partial stats ----------
                partials = []
                for b in range(CB):
                    st6 = _t(setup, [128, 4, 6], F32, f'st6_{b}')
                    xv = xown[b].rearrange("p (a f) -> p a f", f=512)
                    for sg in range(4):
                        nc.vector.bn_stats(out=st6[:, sg, :], in_=xv[:, sg, :])
                    mv = _t(setup, [128, 2], F32, f'mv_{b}')
                    nc.vector.bn_aggr(out=mv, in_=st6)
                    # partial = [sum, sumsq] = [mean*nq, (var+mean^2)*nq]
                    part = _t(setup, [128, 2], F32, f'part_{b}')
                    sq = _t(setup, [128, 1], F32, f'sq_{b}')
                    nc.scalar.activation(out=sq, in_=mv[:, 0:1], func=AF.Square)
                    nc.vector.tensor_tensor(out=sq, in0=sq, in1=mv[:, 1:2],
                                            op=ALU.add)
                    nc.scalar.mul(out=part[:, 0:1], in_=mv[:, 0:1], mul=float(NQ))
                    nc.scalar.mul(out=part[:, 1:2], in_=sq, mul=float(NQ))
                    partials.append(part)

                psg = ps_tr.tile([GROUPS, 2], F32, tag="tr")
                for b in range(CB):
                    nc.tensor.matmul(psg[:, :], gmap[:, b, :], partials[b][:, :],
                                     start=(b == 0), stop=(b == CB - 1))
                part_g = _t(setup, [GROUPS, 2], F32, 'part_g')
                nc.vector.tensor_copy(out=part_g, in_=psg)
                dbg_out('part_g', part_g)

                # ---------------- phase 2: AllReduce ------------------------
                cin = _t(dram_p, [GROUPS, 2], F32, 'cin')
                cout = _t(dram_p, [GROUPS, 2], F32, 'cout')
                gl = _t(setup, [GROUPS, 2], F32, 'gl')
                if "nocoll" in ablate:
                    nc.scalar.mul(out=gl, in_=part_g, mul=float(NC))
                else:
                    nc.gpsimd.dma_start(out=cin[:], in_=part_g)
                    nc.gpsimd.collective_compute(
                        "AllReduce", ALU.add,
                        replica_groups=[list(range(NC))],
                        ins=[cin.opt()], outs=[cout.opt()])
                    nc.gpsimd.dma_start(out=gl, in_=cout[:])
                dbg_out('gl', gl)

                # ---------------- phase 3: stats -> per-channel affine ------
                musd = _t(setup, [GROUPS, 2], F32, 'musd')  # [mu, rstd] per group
                inv_n = 1.0 / float(GN_COUNT)
                nc.scalar.mul(out=musd[:, 0:1], in_=gl[:, 0:1], mul=inv_n)
                m2 = _t(setup, [GROUPS, 1], F32, 'm2')
                nc.scalar.mul(out=m2, in_=gl[:, 1:2], mul=inv_n)
                musq = _t(setup, [GROUPS, 1], F32, 'musq')
                nc.scalar.activation(out=musq, in_=musd[:, 0:1], func=AF.Square)
                nc.vector.tensor_tensor(out=m2, in0=m2, in1=musq, op=ALU.subtract)
                epst = _t(setup, [GROUPS, 1], F32, 'epst')
                nc.vector.memset(epst, EPS)
                nc.scalar.activation(out=m2, in_=m2, func=AF.Sqrt, bias=epst)
                nc.vector.reciprocal(out=musd[:, 1:2], in_=m2)
                dbg_out('musd', musd)

                # scatter group stats to channels; per-channel affine a, b
                a_by_w = {"q": [], "k": [], "v": []}
                bvec16 = []
                for b in range(CB):
                    pssc = ps_tr.tile([128, 2], F32, tag="tr")
                    nc.tensor.matmul(pssc[:, :], gscat[:, b, :], musd[:, :],
                                     start=True, stop=True)
                    mc = _t(setup, [128, 2], F32, f'mc_{b}')
                    nc.vector.tensor_copy(out=mc, in_=pssc)
                    a = _t(setup, [128, 1], F32, f'a_{b}')
                    nc.vector.tensor_tensor(out=a, in0=mc[:, 1:2],
                                            in1=vecs["gamma"][:, b:b + 1],
                                            op=ALU.mult)
                    bb = _t(setup, [128, 1], F32, f'bb_{b}')
                    nc.vector.tensor_tensor(out=bb, in0=mc[:, 0:1], in1=a,
                                            op=ALU.mult)
                    nc.vector.tensor_tensor(out=bb, in0=vecs["beta"][:, b:b + 1],
                                            in1=bb, op=ALU.subtract)
                    bv16 = _t(setup, [128, 1], BF16, f'bv16_{b}')
                    nc.vector.tensor_copy(out=bv16, in_=bb)
                    bvec16.append(bv16)
                    asq = _t(setup, [128, 1], F32, f'asq_{b}')
                    nc.scalar.mul(out=asq, in_=a, mul=scale * RS)
                    ar = _t(setup, [128, 1], F32, f'ar_{b}')
                    nc.scalar.mul(out=ar, in_=a, mul=RS)
                    a_by_w["q"].append(asq)
                    a_by_w["k"].append(ar)
                    a_by_w["v"].append(ar)

                # fold q/k/v weights to fp8 DoubleRow layout: RS * a * wT
                wTp8 = {name: [_t(wfold_p, [128, 2, C], FP8, f'wTp8_{name}{ch}')
                               for ch in range(2)]
                        for name in ("q", "k", "v")}
                for name in ("q", "k", "v"):
                    for b in range(CB):
                        nc.vector.tensor_scalar_mul(
                            wTp8[name][b // 2][:, b % 2, :], wTu[name][b],
                            a_by_w[name][b])

                # folded biases biasF_w[o] = s*RS*((w @ b)[o] + bias_w[o]) from
                # the unfolded bf16 weights (a cancels against b = beta - mu*a)
                biasF = {}
                for name, bvec, s in (("q", "bq", scale * RS),
                                      ("k", "bk", RS), ("v", "bv", 1.0)):
                    bf_t = _t(singles, [128, CB], F32, f'biasF_{name}')
                    for ob in range(CB):
                        psb = ps_tr.tile([128, 1], F32, tag="tr")
                        for b in range(CB):
                            nc.tensor.matmul(
                                psb[:, :],
                                wTu[name][b][:, 128 * ob:128 * (ob + 1)],
                                bvec16[b][:, :],
                                start=(b == 0), stop=(b == CB - 1))
                        nc.vector.tensor_scalar(
                            out=bf_t[:, ob:ob + 1], in0=psb,
                            scalar1=vecs[bvec][:, ob:ob + 1], scalar2=s,
                            op0=ALU.add, op1=ALU.mult)
                    biasF[name] = bf_t

                # v bias folds into the projection bias: since sum_j p_j/d = 1,
                # out = wp@(ov + bias_v) + bp = proj(ov) + (wp@bias_v + bp)
                bvF16 = []
                for b in range(CB):
                    t16 = _t(setup, [128, 1], BF16, f'bvF16_{b}')
                    nc.vector.tensor_copy(out=t16, in_=biasF["v"][:, b:b + 1])
                    bvF16.append(t16)
                biasFP = _t(singles, [128, CB], F32, 'biasFP')
                for ob in range(CB):
                    psb = ps_tr.tile([128, 1], F32, tag="tr")
                    for b in range(CB):
                        nc.tensor.matmul(
                            psb[:, :],
                            wTp["p"][b][:, 128 * ob:128 * (ob + 1)],
                            bvF16[b][:, :],
                            start=(b == 0), stop=(b == CB - 1))
                    nc.vector.tensor_tensor(
                        out=biasFP[:, ob:ob + 1], in0=psb,
                        in1=vecs["bp"][:, ob:ob + 1], op=ALU.add)

                # ---------------- phase 4: K, V^T, Q ------------------------
                K_sb = [_t(k_p, [128, 2, N], FP8, f'K_{oh}')
                        for oh in range(2)]
                for ob in range(CB):
                    for jc in range(N // 512):
                        pk = ps_mm.tile([128, 512], F32, tag="mm")
                        for ch in range(2):
                            nc.tensor.matmul(
                                pk[:, :],
                                wTp8["k"][ch][:, :, 128 * ob:128 * (ob + 1)],
                                x8[ch][:, :, 512 * jc:512 * (jc + 1)],
                                perf_mode=mybir.MatmulPerfMode.DoubleRow,
                                start=(ch == 0), stop=(ch == 1))
                        nc.vector.tensor_scalar_add(
                            K_sb[ob // 2][:, ob % 2, 512 * jc:512 * (jc + 1)],
                            pk, biasF["k"][:, ob:ob + 1])

                Q_sb = [_t(q_p, [128, 2, NQ], FP8, f'Q_{oh}')
                        for oh in range(2)]
                for ob in range(CB):
                    for ic in range(NQ // 512):
                        pq = ps_mm.tile([128, 512], F32, tag="mm")
                        for ch in range(2):
                            nc.tensor.matmul(
                                pq[:, :],
                                wTp8["q"][ch][:, :, 128 * ob:128 * (ob + 1)],
                                x8[ch][:, :, 512 * ic:512 * (ic + 1)],
                                perf_mode=mybir.MatmulPerfMode.DoubleRow,
                                start=(ch == 0), stop=(ch == 1))
                        nc.vector.tensor_scalar_add(
                            Q_sb[ob // 2][:, ob % 2, 512 * ic:512 * (ic + 1)],
                            pq, biasF["q"][:, ob:ob + 1])

                V_sb = [_t(v_p, [128, 2, C], FP8, f'V_{j2}')
                        for j2 in range(N // 256)]
                for jb in range(N // 128):
                    pv = ps_mm.tile([128, 512], F32, tag="mm")
                    for ch in range(2):
                        nc.tensor.matmul(
                            pv[:, :], x8[ch][:, :, 128 * jb:128 * (jb + 1)],
                            wTp8["v"][ch][:, :, :],
                            perf_mode=mybir.MatmulPerfMode.DoubleRow,
                            start=(ch == 0), stop=(ch == 1))
                    nc.vector.tensor_copy(out=V_sb[jb // 2][:, jb % 2, :], in_=pv)


            if "noattn" in ablate:
                for ob in range(CB):
                    nc.sync.dma_start(out=yf[128 * ob:128 * (ob + 1), :],
                                      in_=xown[ob])
                nc.compile_marker = True
            # ---------------- phase 5: attention ----------------------------
            skip_attn = "noattn" in ablate
            with (
                tc.tile_pool(name="attn", bufs=1) as attn_p,
                tc.tile_pool(name="pbuf", bufs=2) as p_pool,
                tc.tile_pool(name="ptbuf", bufs=2) as pt_pool,
                tc.tile_pool(name="obuf", bufs=3) as o_pool,
            ):
                AO = _t(attn_p, [128, CB, NQ], BF16, 'AO')   # attn out (c, i) blocks
                NIB = 0 if skip_attn else NQ // 128      # 16 query blocks
                reps = 4 if "rep4" in ablate else 1
                petr = "dmatr" not in ablate
                for rep, ib in __import__("itertools").product(range(reps), range(NIB)):
                    P_sb = p_pool.tile([128, N], BF16, tag="P")
                    dparts = o_pool.tile([128, N // 1024], F32, tag="dp")
                    for jc4 in range(N // 1024):
                        pss = ps_mm.tile([128, 2, 512], F32, tag="s2", bufs=2)
                        for half in range(2):
                            jc = 2 * jc4 + half
                            for oh in range(2):
                                nc.tensor.matmul(
                                    pss[:, half, :],
                                    Q_sb[oh][:, :, 128 * ib:128 * (ib + 1)],
                                    K_sb[oh][:, :, 512 * jc:512 * (jc + 1)],
                                    perf_mode=mybir.MatmulPerfMode.DoubleRow,
                                    start=(oh == 0), stop=(oh == 1))
                        nc.scalar.activation(
                            out=P_sb[:, 1024 * jc4:1024 * (jc4 + 1)],
                            in_=pss.rearrange("p a b -> p (a b)"),
                            func=AF.Exp, scale=1.0 / (RS * RS),
                            accum_out=dparts[:, jc4:jc4 + 1])
                    dsum = o_pool.tile([128, 1], F32, tag="ds")
                    nc.vector.reduce_sum(out=dsum, in_=dparts,
                                         axis=mybir.AxisListType.X)
                    nc.scalar.mul(out=dsum, in_=dsum, mul=RS)
                    rinv = o_pool.tile([128, 1], F32, tag="ri")
                    nc.vector.reciprocal(out=rinv, in_=dsum)

                    PT8 = pt_pool.tile([128, N // 128, 128], FP8, tag="PT8", bufs=3)
                    if petr:
                        # PE transposes of bf16 P, 8 packed per PSUM bank; the
                        # fp8 cast rides along on the PSUM->SBUF copy
                        for rnd in range(4):
                            ptp = ps_tr.tile([128, 8, 128], BF16, tag="tr")
                            for t8 in range(8):
                                jb = 8 * rnd + t8
                                nc.tensor.matmul(
                                    ptp[:, t8, :],
                                    P_sb[:, 128 * jb:128 * (jb + 1)],
                                    identb[:, :], is_transpose=True)
                            if rnd % 2 == 0:
                                nc.vector.tensor_copy(
                                    out=PT8[:, 8 * rnd:8 * rnd + 8, :], in_=ptp)
                            else:
                                nc.scalar.copy(
                                    out=PT8[:, 8 * rnd:8 * rnd + 8, :], in_=ptp)
                    else:
                        # transpose P in 128x128 blocks on the DMA engines
                        PT = pt_pool.tile([128, N // 128, 128], BF16, tag="PT")
                        for jb in range(N // 128):
                            nc.sync.dma_start(out=PT[:, jb, :],
                                              in_=P_sb[:, 128 * jb:128 * (jb + 1)],
                                              transpose=True)
                        if "dvecast" in ablate:
                            for qt in range(4):
                                nc.vector.tensor_copy(
                                    out=PT8[:, 8 * qt:8 * (qt + 1), :],
                                    in_=PT[:, 8 * qt:8 * (qt + 1), :])
                        else:
                            # cast PT to fp8 on the SWDGE path, in 4 chunks
                            for qt in range(4):
                                nc.gpsimd.dma_start(
                                    out=PT8[:, 8 * qt:8 * (qt + 1), :],
                                    in_=PT[:, 8 * qt:8 * (qt + 1), :])

                    # PV: out^T (i, c) accumulated over j; then scale by 1/d
                    pso = ps_mm.tile([128, 512], F32, tag="mm")
                    NJ2 = N // 256
                    for j2 in range(NJ2):
                        nc.tensor.matmul(pso[:, :],
                                         PT8[:, 2 * j2:2 * j2 + 2, :],
                                         V_sb[j2][:, :, :],
                                         perf_mode=mybir.MatmulPerfMode.DoubleRow,
                                         start=(j2 == 0), stop=(j2 == NJ2 - 1))
                    OT = o_pool.tile([128, C], BF16, tag="OT")
                    nc.vector.tensor_scalar_mul(OT, pso, rinv)

                    if petr:
                        pt2 = ps_tr.tile([128, CB, 128], BF16, tag="tr")
                        for cb in range(CB):
                            nc.tensor.matmul(pt2[:, cb, :],
                                             OT[:, 128 * cb:128 * (cb + 1)],
                                             identb[:, :], is_transpose=True)
                        nc.scalar.copy(out=AO[:, :, 128 * ib:128 * (ib + 1)],
                                       in_=pt2)
                    else:
                        # transpose out^T back to (c, i) into AO via DMA
                        for cb in range(CB):
                            nc.sync.dma_start(
                                out=AO[:, cb, 128 * ib:128 * (ib + 1)],
                                in_=OT[:, 128 * cb:128 * (cb + 1)],
                                transpose=True)

                # ------------- phase 6: proj + residual + store -------------
                for rep, ob in __import__("itertools").product(
                        range(1 if skip_attn else (4 if "rep4" in ablate else 1)),
                        () if skip_attn else range(CB)):
                    for ic in range(NQ // 512):
                        psp = ps_mm.tile([128, 512], F32, tag="mm")
                        for b in range(CB):
                            nc.tensor.matmul(
                                psp[:, :],
                                wTp["p"][b][:, 128 * ob:128 * (ob + 1)],
                                AO[:, b, 512 * ic:512 * (ic + 1)],
                                start=(b == 0), stop=(b == CB - 1))
                        ot = o_pool.tile([128, 512], F32, tag="out")
                        nc.scalar.activation(out=ot, in_=psp, func=AF.Identity,
                                             bias=biasFP[:, ob:ob + 1])
                        nc.vector.tensor_tensor(
                            out=ot, in0=ot,
                            in1=xown[ob][:, 512 * ic:512 * (ic + 1)], op=ALU.add)
                        nc.sync.dma_start(
                            out=yf[128 * ob:128 * (ob + 1),
                                   512 * ic:512 * (ic + 1)],
                            in_=ot)

    nc.compile()
    return nc


def _get_nc(debug=False, ablate=()):
    key = f"nc{int(debug)}{sorted(ablate)}"
    if key not in _CACHED:
        _CACHED[key] = _build(debug, ablate)
    return _CACHED[key]


def _host_inputs(x, gamma, beta, wq, bq, wk, bk, wv, bv, wp, bp):
    gmap = np.zeros((C, GROUPS), dtype=np.float32)
    gmap[np.arange(C), np.arange(C) // (C // GROUPS)] = 1.0
    gscat = np.ascontiguousarray(gmap.T)
    identb = np.eye(128, dtype=ml_dtypes.bfloat16)

    shared = {
        "wq": np.ascontiguousarray(np.asarray(wq, np.float32).astype(ml_dtypes.bfloat16)),
        "wk": np.ascontiguousarray(np.asarray(wk, np.float32).astype(ml_dtypes.bfloat16)),
        "wv": np.ascontiguousarray(np.asarray(wv, np.float32).astype(ml_dtypes.bfloat16)),
        "wp": np.ascontiguousarray(np.asarray(wp, np.float32).astype(ml_dtypes.bfloat16)),
        "gamma": np.ascontiguousarray(gamma, np.float32),
        "beta": np.ascontiguousarray(beta, np.float32),
        "bq": np.ascontiguousarray(bq, np.float32),
        "bk": np.ascontiguousarray(bk, np.float32),
        "bv": np.ascontiguousarray(bv, np.float32),
        "bp": np.ascontiguousarray(bp, np.float32),
        "gmap": gmap, "gscat": gscat, "identb": identb,
    }
    in_maps = []
    for core in range(NC):
        f, h = core // 2, core % 2
        frame = np.asarray(x[0, :, f], dtype=np.float32).reshape(C, N)
        if h == 1:
            frame = np.concatenate([frame[:, NQ:], frame[:, :NQ]], axis=1)
        m = dict(shared)
        m["xb"] = np.ascontiguousarray(frame.astype(ml_dtypes.float8_e4m3))
        m["xh"] = np.ascontiguousarray(frame[:, :NQ])
        in_maps.append(m)
    return in_maps


def _assemble(results):
    y = np.empty((B, C, T, H, W), dtype=np.float32)
    for core in range(NC):
        f, h = core // 2, core % 2
        part = results[core]["yf"].reshape(C, NQ // W, W)
        rows = slice(0, H // 2) if h == 0 else slice(H // 2, H)
        y[0, :, f, rows, :] = part
    return y


def kernel(x, gamma, beta, wq, bq, wk, bk, wv, bv, wp, bp):
    nc = _get_nc()
    in_maps = _host_inputs(x, gamma, beta, wq, bq, wk, bk, wv, bv, wp, bp)
    res = run_bass_kernel_spmd(nc, in_maps, core_ids=list(range(NC)))
    return _assemble(res.results)


# revision 29
# speedup vs baseline: 10.1620x; 1.4107x over previous
"""AttnBlock2D (GroupNorm + QKV 1x1 + full self-attention over N=4096 + proj +
residual) on 8 Trainium2 NeuronCores.

Sharding: data-parallel over the 4 (b*t) frames x 2-way query split within each
frame (core i -> frame i//2, query half i%2).  Each core receives its frame with
tokens rotated so its own query half is tokens [0:2048] (softmax/PV are invariant
to key permutation), so a single uniform SPMD program runs on all 8 cores.

GroupNorm is folded into the QKV weights: hn[c,n] = a_c*x[c,n] + b_c, with the
per-channel affine (a, b) computed from global group stats obtained via a tiny
(32,2) AllReduce of per-core partial sums.  The attention scale C**-0.5 is folded
into wq.  All heavy matmuls run in bf16 with fp32 PSUM accumulation; the residual
add is done in fp32, so bf16 rounding only touches the small attention branch.
"""

import numpy as np
import ml_dtypes

import concourse.bass as bass
import concourse.bacc as bacc
import concourse.mybir as mybir
import concourse.tile as tile
from concourse.bass_utils import run_bass_kernel_spmd

F32 = mybir.dt.float32
BF16 = mybir.dt.bfloat16
FP8 = mybir.dt.float8e4
AF = mybir.ActivationFunctionType
ALU = mybir.AluOpType

# Problem shape (hardcoded per contract)
B, C, T, H, W = 1, 512, 4, 64, 64
N = H * W                # 4096 tokens per frame
GROUPS = 32
EPS = 1e-6
NC = 8                   # cores
NQ = N // 2              # queries per core (2048)
CB = C // 128            # channel blocks (4)
GN_COUNT = (C // GROUPS) * T * N   # elements per group = 16*4*4096

# fp8 weight rescale: folded q/k/v weights (~2e-3) sit below the fp8e4m3
# normal range, so scale them x32 and divide out RS^2=1024 inside the exp
# (S) and RS inside the PV normalization -- exact powers of two.
RS = 32.0

_CACHED = {}


def _t(pool, shape, dtype, nm, bufs=None):
    """pool.tile with name==tag (each call site gets its own persistent slot)."""
    return pool.tile(shape, dtype, name=nm, tag=nm, bufs=bufs)



def _build(debug=False, ablate=()):
    nc = bacc.Bacc(num_devices=NC, name="attnblock2d")
    dbg = {}
    def dbg_out(name, ap):
        if not debug:
            return
        t = nc.dram_tensor(f"dbg_{name}", tuple(ap.shape), ap.dtype,
                           kind="ExternalOutput")
        nc.sync.dma_start(out=t[tuple(slice(0, s) for s in ap.shape)], in_=ap)

    xb_d = nc.dram_tensor("xb", (C, N), FP8, kind="ExternalInput")
    xh_d = nc.dram_tensor("xh", (C, NQ), F32, kind="ExternalInput")
    w_d = {
        "q": nc.dram_tensor("wq", (C, C), BF16, kind="ExternalInput"),
        "k": nc.dram_tensor("wk", (C, C), BF16, kind="ExternalInput"),
        "v": nc.dram_tensor("wv", (C, C), BF16, kind="ExternalInput"),
        "p": nc.dram_tensor("wp", (C, C), BF16, kind="ExternalInput"),
    }
    vec_d = {
        name: nc.dram_tensor(name, (C,), F32, kind="ExternalInput")
        for name in ("gamma", "beta", "bq", "bk", "bv", "bp")
    }
    gmap_d = nc.dram_tensor("gmap", (C, GROUPS), F32, kind="ExternalInput")
    gscat_d = nc.dram_tensor("gscat", (GROUPS, C), F32, kind="ExternalInput")
    identb_d = nc.dram_tensor("identb", (128, 128), BF16, kind="ExternalInput")
    yf = nc.dram_tensor("yf", (C, NQ), F32, kind="ExternalOutput")

    scale = float(C) ** -0.5

    with tile.TileContext(nc) as tc:
        with (
            tc.tile_pool(name="singles", bufs=1) as singles,
            tc.tile_pool(name="xown", bufs=1) as xown_p,
            tc.tile_pool(name="kp", bufs=1) as k_p,
            tc.tile_pool(name="vp", bufs=1) as v_p,
            tc.tile_pool(name="qp", bufs=1) as q_p,
            tc.tile_pool(name="wfold", bufs=1) as wfold_p,
            tc.tile_pool(name="psmm", bufs=2, space="PSUM") as ps_mm,
            tc.tile_pool(name="pstr", bufs=2, space="PSUM") as ps_tr,
            tc.tile_pool(name="dram", bufs=1, space="DRAM") as dram_p,
        ):
            # ---------------- phase 0: input DMAs (critical-path order) -----
            # xown feeds stats -> AllReduce (the longest dependency chain);
            # identb + weights feed the PE transposes that fill the wait.
            xown = [_t(xown_p, [128, NQ], F32, f'xown_{b}') for b in range(CB)]
            for b in range(CB):
                for sg in range(4):
                    nc.sync.dma_start(
                        out=xown[b][:, 512 * sg:512 * (sg + 1)],
                        in_=xh_d[128 * b:128 * (b + 1), 512 * sg:512 * (sg + 1)])

            identb = _t(singles, [128, 128], BF16, 'identb')
            nc.scalar.dma_start(out=identb, in_=identb_d[:, :])
            ident8 = _t(singles, [128, 128], FP8, 'ident8')
            nc.vector.tensor_copy(out=ident8, in_=identb)

            gmap = _t(singles, [128, CB, GROUPS], F32, 'gmap')
            nc.scalar.dma_start(
                out=gmap, in_=gmap_d[:, :].rearrange("(b p) g -> p b g", p=128))
            gscat = _t(singles, [GROUPS, CB, 128], F32, 'gscat')
            nc.scalar.dma_start(
                out=gscat, in_=gscat_d[:, :].rearrange("g (b c) -> g b c", c=128))

            vecs = {}
            for name, ten in vec_d.items():
                t = _t(singles, [128, CB], F32, f'vec_{name}')
                nc.scalar.dma_start(out=t, in_=ten[:].rearrange("(b p) -> p b", p=128))
                vecs[name] = t


            # folded (transposed, bf16) weights live for the whole kernel
            wTp = {
                name: [_t(wfold_p, [128, C], BF16, f'wTp_{name}{b}')
                       for b in range(CB)]
                for name in ("q", "k", "v", "p")
            }

            with (
                tc.tile_pool(name="xb16p", bufs=1) as xb16_p,
                tc.tile_pool(name="setup", bufs=1) as setup,
            ):
                # full frame cast to bf16 (gpsimd casting DMA)
                x8 = [_t(v_p, [128, 2, N], FP8, f'x8_{ch}')
                      for ch in range(2)]
                for ch in range(2):
                    nc.sync.dma_start(
                        out=x8[ch],
                        in_=xb_d[256 * ch:256 * (ch + 1), :].rearrange(
                            "(h p) n -> p h n", p=128))

                # weights (bf16, o rows on partitions), transposed early so
                # the PE does this during the DMA/stats/collective window.
                # NOTE: the rhs of a transpose-mode matmul must be a true
                # identity matrix (its nonzero structure routes the data).
                wTu = {"p": wTp["p"]}
                for name in ("p", "q", "k", "v"):
                    ten = w_d[name]
                    wbig = setup.tile([128, CB, C], BF16, tag="wnat", bufs=2)
                    nc.scalar.dma_start(
                        out=wbig,
                        in_=ten[:, :].rearrange("(b p) c -> p b c", p=128))
                    if name != "p":
                        wTu[name] = [_t(setup, [128, C], BF16, f'wTu_{name}{b}')
                                     for b in range(CB)]
                    for cb in range(CB):
                        pw = ps_tr.tile([128, CB, 128], BF16, tag="tr")
                        for ob in range(CB):
                            nc.tensor.matmul(
                                pw[:, ob, :],
                                wbig[:, ob, 128 * cb:128 * (cb + 1)],
                                identb[:, :], is_transpose=True)
                        nc.scalar.copy(out=wTu[name][cb],
                                       in_=pw.rearrange("p a b -> p (a b)"))

                # ---------------- phase 1: groupnorm partial stats ----------
                partials = []
                for b in range(CB):
                    st6 = _t(setup, [128, 4, 6], F32, f'st6_{b}')
                    xv = xown[b].rearrange("p (a f) -> p a f", f=512)
                    for sg in range(4):
                        nc.vector.bn_stats(out=st6[:, sg, :], in_=xv[:, sg, :])
                    mv = _t(setup, [128, 2], F32, f'mv_{b}')
                    nc.vector.bn_aggr(out=mv, in_=st6)
                    # partial = [sum, sumsq] = [mean*nq, (var+mean^2)*nq]
                    part = _t(setup, [128, 2], F32, f'part_{b}')
                    sq = _t(setup, [128, 1], F32, f'sq_{b}')
                    nc.scalar.activation(out=sq, in_=mv[:, 0:1], func=AF.Square)
                    nc.vector.tensor_tensor(out=sq, in0=sq, in1=mv[:, 1:2],
                                            op=ALU.add)
                    nc.scalar.mul(out=part[:, 0:1], in_=mv[:, 0:1], mul=float(NQ))
                    nc.scalar.mul(out=part[:, 1:2], in_=sq, mul=float(NQ))
                    partials.append(part)

                psg = ps_tr.tile([GROUPS, 2], F32, tag="tr")
                for b in range(CB):
                    nc.tensor.matmul(psg[:, :], gmap[:, b, :], partials[b][:, :],
                                     start=(b == 0), stop=(b == CB - 1))
                part_g = _t(setup, [GROUPS, 2], F32, 'part_g')
                nc.vector.tensor_copy(out=part_g, in_=psg)
                dbg_out('part_g', part_g)

                # ---------------- phase 2: AllReduce ------------------------
                cin = _t(dram_p, [GROUPS, 2], F32, 'cin')
                cout = _t(dram_p, [GROUPS, 2], F32, 'cout')
                gl = _t(setup, [GROUPS, 2], F32, 'gl')
                if "nocoll" in ablate:
                    nc.scalar.mul(out=gl, in_=part_g, mul=float(NC))
                else:
                    nc.gpsimd.dma_start(out=cin[:], in_=part_g)
                    nc.gpsimd.collective_compute(
                        "AllReduce", ALU.add,
                        replica_groups=[list(range(NC))],
                        ins=[cin.opt()], outs=[cout.opt()])
                    nc.gpsimd.dma_start(out=gl, in_=cout[:])
                dbg_out('gl', gl)

                # ---------------- phase 3: stats -> per-channel affine ------
                musd = _t(setup, [GROUPS, 2], F32, 'musd')  # [mu, rstd] per group
                inv_n = 1.0 / float(GN_COUNT)
                nc.scalar.mul(out=musd[:, 0:1], in_=gl[:, 0:1], mul=inv_n)
                m2 = _t(setup, [GROUPS, 1], F32, 'm2')
                nc.scalar.mul(out=m2, in_=gl[:, 1:2], mul=inv_n)
                musq = _t(setup, [GROUPS, 1], F32, 'musq')
                nc.scalar.activation(out=musq, in_=musd[:, 0:1], func=AF.Square)
                nc.vector.tensor_tensor(out=m2, in0=m2, in1=musq, op=ALU.subtract)
                epst = _t(setup, [GROUPS, 1], F32, 'epst')
                nc.vector.memset(epst, EPS)
                nc.scalar.activation(out=m2, in_=m2, func=AF.Sqrt, bias=epst)
                nc.vector.reciprocal(out=musd[:, 1:2], in_=m2)
                dbg_out('musd', musd)

                # scatter group stats to channels; per-channel affine a, b
                a_by_w = {"q": [], "k": [], "v": []}
                bvec16 = []
                for b in range(CB):
                    pssc = ps_tr.tile([128, 2], F32, tag="tr")
                    nc.tensor.matmul(pssc[:, :], gscat[:, b, :], musd[:, :],
                                     start=True, stop=True)
                    mc = _t(setup, [128, 2], F32, f'mc_{b}')
                    nc.vector.tensor_copy(out=mc, in_=pssc)
                    a = _t(setup, [128, 1], F32, f'a_{b}')
                    nc.vector.tensor_tensor(out=a, in0=mc[:, 1:2],
                                            in1=vecs["gamma"][:, b:b + 1],
                                            op=ALU.mult)
                    bb = _t(setup, [128, 1], F32, f'bb_{b}')
                    nc.vector.tensor_tensor(out=bb, in0=mc[:, 0:1], in1=a,
                                            op=ALU.mult)
                    nc.vector.tensor_tensor(out=bb, in0=vecs["beta"][:, b:b + 1],
                                            in1=bb, op=ALU.subtract)
                    bv16 = _t(setup, [128, 1], BF16, f'bv16_{b}')
                    nc.vector.tensor_copy(out=bv16, in_=bb)
                    bvec16.append(bv16)
                    asq = _t(setup, [128, 1], F32, f'asq_{b}')
                    nc.scalar.mul(out=asq, in_=a, mul=scale * RS)
                    ar = _t(setup, [128, 1], F32, f'ar_{b}')
                    nc.scalar.mul(out=ar, in_=a, mul=RS)
                    a_by_w["q"].append(asq)
                    a_by_w["k"].append(ar)
                    a_by_w["v"].append(ar)

                # fold q/k/v weights to fp8 DoubleRow layout: RS * a * wT
                wTp8 = {name: [_t(wfold_p, [128, 2, C], FP8, f'wTp8_{name}{ch}')
                               for ch in range(2)]
                        for name in ("q", "k", "v")}
                for name in ("q", "k", "v"):
                    for b in range(CB):
                        nc.vector.tensor_scalar_mul(
                            wTp8[name][b // 2][:, b % 2, :], wTu[name][b],
                            a_by_w[name][b])

                # folded biases biasF_w[o] = s*RS*((w @ b)[o] + bias_w[o]) from
                # the unfolded bf16 weights (a cancels against b = beta - mu*a)
                biasF = {}
                for name, bvec, s in (("q", "bq", scale * RS),
                                      ("k", "bk", RS), ("v", "bv", 1.0)):
                    bf_t = _t(singles, [128, CB], F32, f'biasF_{name}')
                    for ob in range(CB):
                        psb = ps_tr.tile([128, 1], F32, tag="tr")
                        for b in range(CB):
                            nc.tensor.matmul(
                                psb[:, :],
                                wTu[name][b][:, 128 * ob:128 * (ob + 1)],
                                bvec16[b][:, :],
                                start=(b == 0), stop=(b == CB - 1))
                        nc.vector.tensor_scalar(
                            out=bf_t[:, ob:ob + 1], in0=psb,
                            scalar1=vecs[bvec][:, ob:ob + 1], scalar2=s,
                            op0=ALU.add, op1=ALU.mult)
                    biasF[name] = bf_t

                # v bias folds into the projection bias: since sum_j p_j/d = 1,
                # out = wp@(ov + bias_v) + bp = proj(ov) + (wp@bias_v + bp)
                bvF16 = []
                for b in range(CB):
                    t16 = _t(setup, [128, 1], BF16, f'bvF16_{b}')
                    nc.vector.tensor_copy(out=t16, in_=biasF["v"][:, b:b + 1])
                    bvF16.append(t16)
                biasFP = _t(singles, [128, CB], F32, 'biasFP')
                for ob in range(CB):
                    psb = ps_tr.tile([128, 1], F32, tag="tr")
                    for b in range(CB):
                        nc.tensor.matmul(
                            psb[:, :],
                            wTp["p"][b][:, 128 * ob:128 * (ob + 1)],
                            bvF16[b][:, :],
                            start=(b == 0), stop=(b == CB - 1))
                    nc.vector.tensor_tensor(
                        out=biasFP[:, ob:ob + 1], in0=psb,
                        in1=vecs["bp"][:, ob:ob + 1], op=ALU.add)
                # fold the projection bias into the residual tiles once, so
                # the per-tile ACT bias-add in phase 6 disappears
                for ob in range(CB):
                    nc.vector.tensor_scalar_add(xown[ob], xown[ob],
                                                biasFP[:, ob:ob + 1])

                # ---------------- phase 4: K, V^T, Q ------------------------
                K_sb = [_t(k_p, [128, 2, N], FP8, f'K_{oh}')
                        for oh in range(2)]
                for ob in range(CB):
                    for jc in range(N // 512):
                        pk = ps_mm.tile([128, 512], F32, tag="mm")
                        for ch in range(2):
                            nc.tensor.matmul(
                                pk[:, :],
                                wTp8["k"][ch][:, :, 128 * ob:128 * (ob + 1)],
                                x8[ch][:, :, 512 * jc:512 * (jc + 1)],
                                perf_mode=mybir.MatmulPerfMode.DoubleRow,
                                start=(ch == 0), stop=(ch == 1))
                        nc.vector.tensor_scalar_add(
                            K_sb[ob // 2][:, ob % 2, 512 * jc:512 * (jc + 1)],
                            pk, biasF["k"][:, ob:ob + 1])

                Q_sb = [_t(q_p, [128, 2, NQ], FP8, f'Q_{oh}')
                        for oh in range(2)]
                for ob in range(CB):
                    for ic in range(NQ // 512):
                        pq = ps_mm.tile([128, 512], F32, tag="mm")
                        for ch in range(2):
                            nc.tensor.matmul(
                                pq[:, :],
                                wTp8["q"][ch][:, :, 128 * ob:128 * (ob + 1)],
                                x8[ch][:, :, 512 * ic:512 * (ic + 1)],
                                perf_mode=mybir.MatmulPerfMode.DoubleRow,
                                start=(ch == 0), stop=(ch == 1))
                        nc.vector.tensor_scalar_add(
                            Q_sb[ob // 2][:, ob % 2, 512 * ic:512 * (ic + 1)],
                            pq, biasF["q"][:, ob:ob + 1])



            if "noattn" in ablate:
                for ob in range(CB):
                    nc.sync.dma_start(out=yf[128 * ob:128 * (ob + 1), :],
                                      in_=xown[ob])
                nc.compile_marker = True
            # ---------------- phase 5: attention ----------------------------
            skip_attn = "noattn" in ablate
            with (
                tc.tile_pool(name="attn", bufs=1) as attn_p,
                tc.tile_pool(name="pbuf", bufs=2) as p_pool,
                tc.tile_pool(name="ptbuf", bufs=2) as pt_pool,
                tc.tile_pool(name="obuf", bufs=3) as o_pool,
            ):
                AO = _t(attn_p, [128, CB, NQ], BF16, 'AO')   # attn out (c, i) blocks
                NIB = 0 if skip_attn else NQ // 128      # 16 query blocks
                reps = 4 if "rep4" in ablate else 1
                petr = "dmatr" not in ablate
                for rep, ib in __import__("itertools").product(range(reps), range(NIB)):
                    P_sb = p_pool.tile([128, N], BF16, tag="P")
                    dparts = o_pool.tile([128, N // 1024], F32, tag="dp")
                    for jc4 in range(N // 1024):
                        pss = ps_mm.tile([128, 2, 512], F32, tag="s2", bufs=2)
                        for half in range(2):
                            jc = 2 * jc4 + half
                            for oh in range(2):
                                nc.tensor.matmul(
                                    pss[:, half, :],
                                    Q_sb[oh][:, :, 128 * ib:128 * (ib + 1)],
                                    K_sb[oh][:, :, 512 * jc:512 * (jc + 1)],
                                    perf_mode=mybir.MatmulPerfMode.DoubleRow,
                                    start=(oh == 0), stop=(oh == 1))
                        nc.scalar.activation(
                            out=P_sb[:, 1024 * jc4:1024 * (jc4 + 1)],
                            in_=pss.rearrange("p a b -> p (a b)"),
                            func=AF.Exp, scale=1.0 / (RS * RS),
                            accum_out=dparts[:, jc4:jc4 + 1])
                    if rep == 0 and ib == 0:
                        # V production overlaps ib0's exp on the ACT engine
                        V_sb = [_t(v_p, [128, 2, C], FP8, f'V_{j2}')
                                for j2 in range(N // 256)]
                        for jb in range(N // 128):
                            pv = ps_mm.tile([128, 512], F32, tag="mm")
                            for ch in range(2):
                                nc.tensor.matmul(
                                    pv[:, :],
                                    x8[ch][:, :, 128 * jb:128 * (jb + 1)],
                                    wTp8["v"][ch][:, :, :],
                                    perf_mode=mybir.MatmulPerfMode.DoubleRow,
                                    start=(ch == 0), stop=(ch == 1))
                            nc.vector.tensor_copy(out=V_sb[jb // 2][:, jb % 2, :],
                                                  in_=pv)
                    dsum = o_pool.tile([128, 1], F32, tag="ds")
                    nc.vector.reduce_sum(out=dsum, in_=dparts,
                                         axis=mybir.AxisListType.X)
                    nc.scalar.mul(out=dsum, in_=dsum, mul=RS)
                    rinv = o_pool.tile([128, 1], F32, tag="ri")
                    nc.vector.reciprocal(out=rinv, in_=dsum)

                    PT8 = pt_pool.tile([128, N // 128, 128], FP8, tag="PT8", bufs=3)
                    if petr:
                        # PE transposes of bf16 P, 8 packed per PSUM bank; the
                        # fp8 cast rides along on the PSUM->SBUF copy
                        for rnd in range(4):
                            ptp = ps_tr.tile([128, 8, 128], BF16, tag="tr")
                            for t8 in range(8):
                                jb = 8 * rnd + t8
                                nc.tensor.matmul(
                                    ptp[:, t8, :],
                                    P_sb[:, 128 * jb:128 * (jb + 1)],
                                    identb[:, :], is_transpose=True)
                            if rnd % 2 == 0:
                                nc.vector.tensor_copy(
                                    out=PT8[:, 8 * rnd:8 * rnd + 8, :], in_=ptp)
                            else:
                                nc.scalar.copy(
                                    out=PT8[:, 8 * rnd:8 * rnd + 8, :], in_=ptp)
                    else:
                        # transpose P in 128x128 blocks on the DMA engines
                        PT = pt_pool.tile([128, N // 128, 128], BF16, tag="PT")
                        for jb in range(N // 128):
                            nc.sync.dma_start(out=PT[:, jb, :],
                                              in_=P_sb[:, 128 * jb:128 * (jb + 1)],
                                              transpose=True)
                        if "dvecast" in ablate:
                            for qt in range(4):
                                nc.vector.tensor_copy(
                                    out=PT8[:, 8 * qt:8 * (qt + 1), :],
                                    in_=PT[:, 8 * qt:8 * (qt + 1), :])
                        else:
                            # cast PT to fp8 on the SWDGE path, in 4 chunks
                            for qt in range(4):
                                nc.gpsimd.dma_start(
                                    out=PT8[:, 8 * qt:8 * (qt + 1), :],
                                    in_=PT[:, 8 * qt:8 * (qt + 1), :])

                    # PV: out^T (i, c) accumulated over j; then scale by 1/d
                    pso = ps_mm.tile([128, 512], F32, tag="mm")
                    NJ2 = N // 256
                    for j2 in range(NJ2):
                        nc.tensor.matmul(pso[:, :],
                                         PT8[:, 2 * j2:2 * j2 + 2, :],
                                         V_sb[j2][:, :, :],
                                         perf_mode=mybir.MatmulPerfMode.DoubleRow,
                                         start=(j2 == 0), stop=(j2 == NJ2 - 1))
                    OT = o_pool.tile([128, C], BF16, tag="OT")
                    nc.vector.tensor_scalar_mul(OT, pso, rinv)

                    if petr:
                        pt2 = ps_tr.tile([128, CB, 128], BF16, tag="tr")
                        for cb in range(CB):
                            nc.tensor.matmul(pt2[:, cb, :],
                                             OT[:, 128 * cb:128 * (cb + 1)],
                                             identb[:, :], is_transpose=True)
                        nc.scalar.copy(out=AO[:, :, 128 * ib:128 * (ib + 1)],
                                       in_=pt2)
                    else:
                        # transpose out^T back to (c, i) into AO via DMA
                        for cb in range(CB):
                            nc.sync.dma_start(
                                out=AO[:, cb, 128 * ib:128 * (ib + 1)],
                                in_=OT[:, 128 * cb:128 * (cb + 1)],
                                transpose=True)

                # ------------- phase 6: proj + residual + store -------------
                for rep, ob in __import__("itertools").product(
                        range(1 if skip_attn else (4 if "rep4" in ablate else 1)),
                        () if skip_attn else range(CB)):
                    for ic in range(NQ // 512):
                        psp = ps_mm.tile([128, 512], F32, tag="mm")
                        for b in range(CB):
                            nc.tensor.matmul(
                                psp[:, :],
                                wTp["p"][b][:, 128 * ob:128 * (ob + 1)],
                                AO[:, b, 512 * ic:512 * (ic + 1)],
                                start=(b == 0), stop=(b == CB - 1))
                        ot = o_pool.tile([128, 512], F32, tag="out")
                        nc.vector.tensor_tensor(
                            out=ot, in0=psp,
                            in1=xown[ob][:, 512 * ic:512 * (ic + 1)], op=ALU.add)
                        nc.sync.dma_start(
                            out=yf[128 * ob:128 * (ob + 1),
                                   512 * ic:512 * (ic + 1)],
                            in_=ot)

    nc.compile()
    return nc


def _get_nc(debug=False, ablate=()):
    key = f"nc{int(debug)}{sorted(ablate)}"
    if key not in _CACHED:
        _CACHED[key] = _build(debug, ablate)
    return _CACHED[key]


def _host_inputs(x, gamma, beta, wq, bq, wk, bk, wv, bv, wp, bp):
    gmap = np.zeros((C, GROUPS), dtype=np.float32)
    gmap[np.arange(C), np.arange(C) // (C // GROUPS)] = 1.0
    gscat = np.ascontiguousarray(gmap.T)
    identb = np.eye(128, dtype=ml_dtypes.bfloat16)

    shared = {
        "wq": np.ascontiguousarray(np.asarray(wq, np.float32).astype(ml_dtypes.bfloat16)),
        "wk": np.ascontiguousarray(np.asarray(wk, np.float32).astype(ml_dtypes.bfloat16)),
        "wv": np.ascontiguousarray(np.asarray(wv, np.float32).astype(ml_dtypes.bfloat16)),
        "wp": np.ascontiguousarray(np.asarray(wp, np.float32).astype(ml_dtypes.bfloat16)),
        "gamma": np.ascontiguousarray(gamma, np.float32),
        "beta": np.ascontiguousarray(beta, np.float32),
        "bq": np.ascontiguousarray(bq, np.float32),
        "bk": np.ascontiguousarray(bk, np.float32),
        "bv": np.ascontiguousarray(bv, np.float32),
        "bp": np.ascontiguousarray(bp, np.float32),
        "gmap": gmap, "gscat": gscat, "identb": identb,
    }
    in_maps = []
    for core in range(NC):
        f, h = core // 2, core % 2
        frame = np.asarray(x[0, :, f], dtype=np.float32).reshape(C, N)
        if h == 1:
            frame = np.concatenate([frame[:, NQ:], frame[:, :NQ]], axis=1)
        m = dict(shared)
        m["xb"] = np.ascontiguousarray(frame.astype(ml_dtypes.float8_e4m3))
        m["xh"] = np.ascontiguousarray(frame[:, :NQ])
        in_maps.append(m)
    return in_maps


def _assemble(results):
    y = np.empty((B, C, T, H, W), dtype=np.float32)
    for core in range(NC):
        f, h = core // 2, core % 2
        part = results[core]["yf"].reshape(C, NQ // W, W)
        rows = slice(0, H // 2) if h == 0 else slice(H // 2, H)
        y[0, :, f, rows, :] = part
    return y


def kernel(x, gamma, beta, wq, bq, wk, bk, wv, bv, wp, bp):
    nc = _get_nc()
    in_maps = _host_inputs(x, gamma, beta, wq, bq, wk, bk, wv, bv, wp, bp)
    res = run_bass_kernel_spmd(nc, in_maps, core_ids=list(range(NC)))
    return _assemble(res.results)
